# revision 13
# baseline (speedup 1.0000x reference)
"""Trainium2 Bass kernel for nn_Attention_Layer_78855599554595.

GQA attention layer: QKV proj -> causal GQA attention (16 heads, 4 kv heads,
E=128) -> out proj -> exact GELU -> residual -> LayerNorm.  B=2, L=2048, D=2048.

Sharding: zero-communication interleaved sequence parallelism.
  - 8 cores = 2 batches x 4 cores/batch.
  - Core j of a batch owns query rows in g=64-row blocks strided by 4:
    global blocks {j, j+4, ..., j+28} (512 rows).  For key block kb
    (256 keys), query-buffer columns [64*kb, 512) attend to it; the
    64-wide window at the start is the diagonal (host-supplied 0/1
    multiplicative mask, identical for every kb).
  - Each core computes K/V for its full batch (redundant 4x; on-chip
    collectives are slower than the 55us of PE time they would save).

v3 design (v2's trace showed the PE starving during the scalar-bound
attention phase, HAM-throttling to 1.2GHz; fixed by interleaving):
  - Q-projection of heads 8-15 is interleaved into heads 0-7's
    attention; out-proj partial sums over heads 0-7 are interleaved
    into heads 8-15's attention.  The PE never idles, stays at 2.4GHz,
    and half the out-projection rides under the exp shadow.
  - The causal mask is an additive -1e9 add on the vector engine on
    the PSUM scores (pre-exp), off the slow gpsimd path.

v2 design (from trace analysis of the 542us v1):
  - All matmuls run bf16 x bf16 (fp32 PSUM accumulate).  Host stages
    x^T and all weights as bf16: halves DMA, kills every fp32->fp32r
    tensor_copy, and removes the fp32r small-free-dim 4x penalty so the
    causal q-window never needs clamping.
  - K^T is produced directly by the projection matmul (Wk tile
    stationary, x^T moving) - no PE transposes, no scalar evictions.
  - K/V projection streams kt (contraction) innermost in 4 row-groups:
    first matmul needs only 1 weight tile + 1 x tile (~2us), not 8MB.
  - Softmax: exp in one scalar.activation per (head, key block) over
    both key subtiles ([128,2,qc] PSUM read); causal mask applied
    after exp as a 0/1 multiply on the gpsimd engine (SBUF-only);
    1/l broadcast via gpsimd.partition_broadcast (no PE broadcast
    matmul, no scalar copy).
  - Out-proj + GELU + residual + LN stats (vector bn_stats/bn_aggr)
    are fused per (oc, rt) tile; rstd via a vector-only Newton rsqrt
    (no scalar Sqrt -> no ACT table-set ping-pong); the LayerNorm +
    output DMA for each row tile drains right after its last column
    chunk, eliminating the 40us serial tail.

Host-side (free): transposes/gathers, bf16 staging, mask construction.
"""

import sys

sys.path.insert(0, "/opt/trn_rl_repo")

import numpy as np

from contextlib import ExitStack
from dataclasses import dataclass

from concourse import bacc, mybir, tile

F32 = mybir.dt.float32
R = mybir.dt.float32r
BF = mybir.dt.bfloat16
AF = mybir.ActivationFunctionType
ALU = mybir.AluOpType


@dataclass(frozen=True)
class Cfg:
    L: int = 2048          # sequence length (per batch)
    D: int = 2048          # model dim
    H: int = 16            # query heads
    KV: int = 4            # kv heads
    E: int = 128           # head dim (= partition width)
    trivial_affine: bool = False  # gamma==1, beta==0, bo==0: skip those ops
    act: object = None     # None -> exact GELU (CoreSim lacks Gelu; tests
                           # may override with an implemented fn like Tanh)

    @property
    def g(self):           # q block granularity (32 blocks across L)
        return self.L // 32

    @property
    def KB(self):          # key block size
        return self.L // 8

    @property
    def KSS(self):         # key subtile (partition) size
        return min(self.KB, 128)

    @property
    def ST(self):          # key subtiles per key block
        return max(1, self.KB // 128)

    @property
    def QR(self):          # query rows per core
        return self.L // 4

    @property
    def KT(self):          # contraction tiles over D
        return self.D // 128

    @property
    def RT(self):          # 128-row tiles of the core's q rows
        return self.QR // 128

    @property
    def OC(self):          # out-proj / LN column chunk
        return min(self.D, 512)


def build_program(cfg: Cfg):
    """Build the single-core SPMD Bass program. Returns finalized nc."""
    L, D, H, KV, E = cfg.L, cfg.D, cfg.H, cfg.KV, cfg.E
    g, KSS, ST, QR, KT, RT, OC = (cfg.g, cfg.KSS, cfg.ST, cfg.QR, cfg.KT,
                                  cfg.RT, cfg.OC)
    NOC = D // OC
    NKB = L // cfg.KB      # 8 key blocks
    inv_sqrt_e = 1.0 / float(np.sqrt(E))
    act_fn = cfg.act if cfg.act is not None else AF.Gelu

    nc = bacc.Bacc(None, target_bir_lowering=False)

    # ---- DRAM I/O (per-core data; same names on every core) ----
    xtb = nc.dram_tensor("xtb", [D, L], R, kind="ExternalInput")      # x[b].T
    xtqb = nc.dram_tensor("xtqb", [D, QR], BF, kind="ExternalInput")  # q cols
    xq = nc.dram_tensor("xq", [QR, D], F32, kind="ExternalInput")     # residual
    wqb = nc.dram_tensor("wqb", [D, H * E], BF, kind="ExternalInput")
    wkb = nc.dram_tensor("wkb", [D, KV * E], R, kind="ExternalInput")
    wvb = nc.dram_tensor("wvb", [D, KV * E], R, kind="ExternalInput")
    wob = nc.dram_tensor("wob", [H * E, D], BF, kind="ExternalInput")
    bqT = nc.dram_tensor("bqT", [E, H], F32, kind="ExternalInput")
    bkT = nc.dram_tensor("bkT", [E, KV], F32, kind="ExternalInput")
    bvb = nc.dram_tensor("bvb", [128, KV * E], F32, kind="ExternalInput")
    # additive diagonal mask (0 / -1e9) in S^T layout: [key (2x128), q (64)]
    mk01 = nc.dram_tensor("mk01", [cfg.KB, g], F32, kind="ExternalInput")
    if not cfg.trivial_affine:
        bob = nc.dram_tensor("bob", [128, D], F32, kind="ExternalInput")
        gmb = nc.dram_tensor("gmb", [128, D], F32, kind="ExternalInput")
        btb = nc.dram_tensor("btb", [128, D], F32, kind="ExternalInput")
    out = nc.dram_tensor("out", [QR, D], F32, kind="ExternalOutput")

    with tile.TileContext(nc) as tc, ExitStack() as top:
        const = top.enter_context(tc.tile_pool(name="const", bufs=1))
        ctxp = top.enter_context(tc.tile_pool(name="ctxp", bufs=1))
        pap = top.enter_context(tc.tile_pool(name="pap", bufs=1))
        kvq_stack = ExitStack()
        kvq = kvq_stack.enter_context(tc.tile_pool(name="kvq", bufs=1))
        qtp = kvq_stack.enter_context(tc.tile_pool(name="qtp", bufs=1))

        # ---- constants (vector DMA queue; issued at t=0) ----
        ones_kb = const.tile([128, 2], BF)
        nc.gpsimd.memset(ones_kb[:], 1.0)
        bq_t = const.tile([E, H], F32)
        bk_t = const.tile([E, KV], F32)
        bv_t = const.tile([128, KV * E], F32)
        mk_t = const.tile([KSS, ST, g], F32)
        nc.gpsimd.dma_start(out=bq_t[:], in_=bqT[:])
        nc.gpsimd.dma_start(out=bk_t[:], in_=bkT[:])
        nc.gpsimd.dma_start(out=bv_t[:], in_=bvb[:])
        nc.gpsimd.dma_start(out=mk_t[:],
                            in_=mk01.rearrange("(s p) q -> p s q", p=KSS))

        # persistent activations
        kT = [kvq.tile([E, L], BF, tag=f"kT{kv}", name=f"kT{kv}")
              for kv in range(KV)]
        vN = kvq.tile([KSS, L // KSS, KV * E], BF)      # V natural, key-major
        qT = [qtp.tile([E, QR], BF, tag=f"qT{h}", name=f"qT{h}")
              for h in range(H)]
        ctxT = [ctxp.tile([E, QR], BF, tag=f"cT{h}", name=f"cT{h}")
                for h in range(H)]
        # out-proj partial sums over heads 0..7 (filled during phase 3b)
        pA = [pap.tile([128, D], F32, tag=f"pA{rt}", name=f"pA{rt}")
              for rt in range(RT)]

        # phase-2 moving operand: resident x^T at q rows (vector queue, t=0)
        xtq_stack = ExitStack()
        xtqp = xtq_stack.enter_context(tc.tile_pool(name="xtqp", bufs=1))
        xtq_r = xtqp.tile([128, KT, QR], BF)
        for kt in range(KT):
            nc.scalar.dma_start(out=xtq_r[:, kt, :],
                                in_=xtqb[kt * 128:(kt + 1) * 128, :])

        # ================= Phase 1: K/V projections ========================
        # kT direct:  out[E, rows] += Wk_tile.T @ xT_tile   (moving = x^T)
        # V natural:  out[rows, kvE] += xT_tile.T @ Wv_tile (moving = Wv)
        with ExitStack() as ph:
            wkv = ph.enter_context(tc.tile_pool(name="wkv", bufs=1))
            stage = ph.enter_context(tc.tile_pool(name="stage1", bufs=4))
            ps1 = ph.enter_context(tc.tile_pool(name="ps1", bufs=1,
                                                space="PSUM"))
            wk_r = wkv.tile([128, KT, KV * E], R)
            wv_r = wkv.tile([128, KT, KV * E], R)
            for kt in range(KT):
                nc.sync.dma_start(out=wk_r[:, kt, :],
                                  in_=wkb[kt * 128:(kt + 1) * 128, :])
                nc.sync.dma_start(out=wv_r[:, kt, :],
                                  in_=wvb[kt * 128:(kt + 1) * 128, :])

            NG = L // 512
            for g4 in range(NG):
                c0 = 512 * g4
                pKT = [ps1.tile([E, 512], F32, tag=f"pKT{kv}",
                                name=f"pKT{kv}") for kv in range(KV)]
                pV = [ps1.tile([128, KV * E], F32, tag=f"pV{c}",
                               name=f"pV{c}") for c in range(4)]
                for kt in range(KT):
                    xg = stage.tile([128, 512], R, tag="xg")
                    eng = nc.gpsimd if kt % 2 == 0 else nc.sync
                    eng.dma_start(
                        out=xg[:], in_=xtb[kt * 128:(kt + 1) * 128,
                                           c0:c0 + 512])
                    for kv in range(KV):
                        nc.tensor.matmul(
                            pKT[kv][:], wk_r[:, kt, kv * E:(kv + 1) * E],
                            xg[:], start=(kt == 0), stop=(kt == KT - 1))
                    for c in range(4):
                        nc.tensor.matmul(
                            pV[c][:], xg[:, c * 128:(c + 1) * 128],
                            wv_r[:, kt, :], start=(kt == 0),
                            stop=(kt == KT - 1))
                for kv in range(KV):
                    nc.vector.tensor_scalar(
                        kT[kv][:, c0:c0 + 512], pKT[kv][:],
                        bk_t[:, kv:kv + 1], None, op0=ALU.add)
                for c in range(4):
                    nc.vector.tensor_add(vN[:, g4 * 4 + c, :], pV[c][:],
                                         bv_t[:])

        # ====== Phase 2a: Q^T projection, heads 0..7 =======================
        # (heads 8..15 are projected inside phase 3a as PE filler work)
        with ExitStack() as ph:
            stage = ph.enter_context(tc.tile_pool(name="stage2", bufs=8))
            ps2 = ph.enter_context(tc.tile_pool(name="ps2", bufs=1,
                                                space="PSUM"))
            HB = 8
            pqs = [ps2.tile([E, QR], F32, tag=f"pq{hh}", name=f"pq{hh}")
                   for hh in range(HB)]
            for kt in range(KT):
                wqs = stage.tile([128, HB, E], BF, tag="wqs")
                nc.sync.dma_start(
                    out=wqs[:],
                    in_=wqb[kt * 128:(kt + 1) * 128, 0:HB * E]
                    .rearrange("p (h e) -> p h e", h=HB))
                for hh in range(HB):
                    nc.tensor.matmul(
                        pqs[hh][:], wqs[:, hh, :], xtq_r[:, kt, :],
                        start=(kt == 0), stop=(kt == KT - 1))
            for hh in range(HB):
                nc.vector.tensor_scalar(
                    qT[hh][:], pqs[hh][:], bq_t[:, hh:hh + 1], None,
                    op0=ALU.add)

        # ================= Phase 3: attention ==============================
        # The exp stream on the scalar engine is the phase bottleneck, so
        # the PE is kept busy (and HAM-warm) with interleaved filler:
        #   heads 0..7:  Q-projection of head 8+h (2 matmuls / kb slot)
        #   heads 8..15: out-proj partial sums over heads 0..7 into pA
        with ExitStack() as ph:
            ps_s = ph.enter_context(tc.tile_pool(name="pss", bufs=2,
                                                 space="PSUM"))
            ps_c = ph.enter_context(tc.tile_pool(name="psc", bufs=2,
                                                 space="PSUM"))
            ps_l = ph.enter_context(tc.tile_pool(name="psl", bufs=1,
                                                 space="PSUM"))
            ps_f = ph.enter_context(tc.tile_pool(name="psf", bufs=1,
                                                 space="PSUM"))
            exp_p = ph.enter_context(tc.tile_pool(name="expp", bufs=3))
            lso = ph.enter_context(tc.tile_pool(name="lso", bufs=2))
            stage3 = ph.enter_context(tc.tile_pool(name="stage3", bufs=2))
            woAp = ph.enter_context(tc.tile_pool(name="woAp", bufs=1))

            # wo rows for heads 0..7 (the 3b filler's moving operand)
            woA = woAp.tile([128, H // 2, D], BF)
            for hh in range(H // 2):
                nc.sync.dma_start(out=woA[:, hh, :],
                                  in_=wob[hh * E:(hh + 1) * E, :])

            wq2 = {}

            def emit_wq2_dma(h2):
                if h2 >= H:
                    return
                w = stage3.tile([128, KT, E], BF, tag="wq2",
                                name=f"wq2_{h2}")
                nc.sync.dma_start(
                    out=w[:],
                    in_=wqb[:, h2 * E:(h2 + 1) * E]
                    .rearrange("(kt p) e -> p kt e", p=128))
                wq2[h2] = w

            emit_wq2_dma(8)
            fill_state = {"pq2": None, "pys": None, "kt": 0, "gs": 0}
            # filler matmuls per kb slot, weighted toward the small-qc
            # (PE-light) late key blocks so exp latency stays hidden
            FILL_W = [1, 1, 1, 2, 2, 2, 3, 4]
            assert sum(FILL_W) == 16

            def emit_filler(h, kb):
                n = FILL_W[kb]
                if h < H // 2:
                    h2 = 8 + h
                    if kb == 0:
                        fill_state["pq2"] = ps_f.tile([E, QR], F32,
                                                      tag="fill",
                                                      name=f"pq2_{h2}")
                        fill_state["kt"] = 0
                    pq2 = fill_state["pq2"]
                    for _ in range(n):
                        kt = fill_state["kt"]
                        fill_state["kt"] += 1
                        nc.tensor.matmul(
                            pq2[:], wq2[h2][:, kt, :], xtq_r[:, kt, :],
                            start=(kt == 0), stop=(kt == KT - 1))
                    if kb == NKB - 1:
                        nc.vector.tensor_scalar(
                            qT[h2][:], pq2[:], bq_t[:, h2:h2 + 1], None,
                            op0=ALU.add)
                else:
                    for _ in range(n):
                        gs = fill_state["gs"]
                        fill_state["gs"] += 1
                        u, step = divmod(gs, 8)
                        oc, rt = divmod(u, RT)
                        if step == 0:
                            fill_state["pys"] = ps_f.tile([128, OC], F32,
                                                          tag="fill",
                                                          name=f"pysA_{u}")
                        pys = fill_state["pys"]
                        nc.tensor.matmul(
                            pys[:], ctxT[step][:, rt * 128:(rt + 1) * 128],
                            woA[:, step, oc * OC:(oc + 1) * OC],
                            start=(step == 0), stop=(step == 7))
                        if step == 7:
                            nc.vector.tensor_copy(
                                pA[rt][:, oc * OC:(oc + 1) * OC], pys[:])

            for h in range(H):
                kv = h % KV
                if h < H // 2:
                    emit_wq2_dma(9 + h)
                pctx = ps_c.tile([E, QR], F32, tag="pctx")
                pl = ps_l.tile([2, QR], F32, tag="pl")
                eSs = [None] * NKB
                q0s = [None] * NKB

                def emit_pl_ctx(kb):
                    eS, q0 = eSs[kb], q0s[kb]
                    qc = QR - q0
                    first = (kb == 0)
                    last = (kb == NKB - 1)
                    for st in range(ST):
                        k0 = (kb * ST + st)
                        nc.tensor.matmul(
                            pl[:, q0:], ones_kb[:, :], eS[:, st, :qc],
                            start=first and st == 0,
                            stop=last and st == ST - 1,
                            skip_group_check=True)
                        nc.tensor.matmul(
                            pctx[:, q0:], vN[:, k0, kv * E:(kv + 1) * E],
                            eS[:, st, :qc],
                            start=first and st == 0,
                            stop=last and st == ST - 1,
                            skip_group_check=True)

                for kb in range(NKB):
                    q0 = g * kb
                    qc = QR - q0
                    q0s[kb] = q0
                    pS = ps_s.tile([KSS, ST, QR], F32, tag="pS")
                    for st in range(ST):
                        k0 = kb * cfg.KB + st * KSS
                        nc.tensor.matmul(pS[:, st, :qc],
                                         kT[kv][:, k0:k0 + KSS],
                                         qT[h][:, q0:], start=True, stop=True)
                    emit_filler(h, kb)
                    # pipeline: previous block's pl/pctx go behind these
                    # scores so the PE isn't blocked on this block's exp.
                    if kb > 0:
                        emit_pl_ctx(kb - 1)
                    # additive causal mask (0/-1e9) on the diagonal window
                    nc.vector.tensor_add(pS[:, :, :g], pS[:, :, :g], mk_t[:])
                    eS = exp_p.tile([KSS, ST, QR], BF, tag="eS")
                    nc.scalar.activation(eS[:, :, :qc], pS[:, :, :qc],
                                         AF.Exp, scale=inv_sqrt_e)
                    eSs[kb] = eS
                emit_pl_ctx(NKB - 1)

                l2f = lso.tile([1, QR], F32, tag="l2f")
                nc.vector.reciprocal_approx_fast(l2f[:], pl[:1, :])
                rb = lso.tile([128, QR], F32, tag="rb")
                nc.gpsimd.partition_broadcast(rb[:], l2f[:])
                nc.vector.tensor_mul(ctxT[h][:], pctx[:], rb[:])

        xtq_stack.close()
        kvq_stack.close()

        # ===== Phase 4+5: out-proj (heads 8..15) + GELU + residual + LN ====
        # rt-outer with wo fully resident: each row tile's LayerNorm +
        # output DMA drains while the next row tile's matmuls run.
        with ExitStack() as ph:
            wop = ph.enter_context(tc.tile_pool(name="wop", bufs=1))
            rfp = ph.enter_context(tc.tile_pool(name="rfp", bufs=2))
            ps_y = ph.enter_context(tc.tile_pool(name="psy", bufs=4,
                                                 space="PSUM"))
            ep = ph.enter_context(tc.tile_pool(name="epp", bufs=3))
            stat = ph.enter_context(tc.tile_pool(name="stat", bufs=1))
            gbp = ph.enter_context(tc.tile_pool(name="gbp", bufs=1))

            woB = wop.tile([128, H // 2, D], BF)
            for hh in range(H // 2):
                h2 = H // 2 + hh
                nc.sync.dma_start(out=woB[:, hh, :],
                                  in_=wob[h2 * E:(h2 + 1) * E, :])
            if not cfg.trivial_affine:
                bo_f = gbp.tile([128, D], F32)
                gm_f = gbp.tile([128, D], F32)
                bt_f = gbp.tile([128, D], F32)
                nc.scalar.dma_start(out=bo_f[:], in_=bob[:])
                nc.scalar.dma_start(out=gm_f[:], in_=gmb[:])
                nc.scalar.dma_start(out=bt_f[:], in_=btb[:])

            for rt in range(RT):
                r_full = rfp.tile([128, D], F32, tag="rf")
                bna = stat.tile([128, NOC, 6], F32, tag="bna")
                for oc in range(NOC):
                    pys = ps_y.tile([128, OC], F32, tag="pys")
                    for hh in range(H // 2):
                        nc.tensor.matmul(
                            pys[:],
                            ctxT[H // 2 + hh][:, rt * 128:(rt + 1) * 128],
                            woB[:, hh, oc * OC:(oc + 1) * OC],
                            start=(hh == 0), stop=(hh == H // 2 - 1))
                    tb = ep.tile([128, OC], F32, tag="tb")
                    nc.vector.tensor_add(tb[:], pys[:],
                                         pA[rt][:, oc * OC:(oc + 1) * OC])
                    if not cfg.trivial_affine:
                        tb2 = ep.tile([128, OC], F32, tag="tb2")
                        nc.vector.tensor_add(
                            tb2[:], tb[:], bo_f[:, oc * OC:(oc + 1) * OC])
                        tb = tb2
                    t2 = ep.tile([128, OC], F32, tag="t2")
                    nc.scalar.activation(t2[:], tb[:], act_fn)
                    xqt = ep.tile([128, OC], F32, tag="xqt")
                    nc.scalar.dma_start(
                        out=xqt[:],
                        in_=xq[rt * 128:(rt + 1) * 128, oc * OC:(oc + 1) * OC])
                    rch = r_full[:, oc * OC:(oc + 1) * OC]
                    nc.vector.tensor_add(rch, t2[:], xqt[:])
                    nc.vector.bn_stats(bna[:, oc, :], rch)
                # stats complete for this row tile: LN + drain now, while
                # the next row tile's matmuls occupy the PE.
                mv = stat.tile([128, 2], F32, tag="mv")
                nc.vector.bn_aggr(mv[:], bna[:])
                v_e = stat.tile([128, 1], F32, tag="ve")
                nc.vector.tensor_scalar_add(v_e[:], mv[:, 1:2], 1e-5)
                # Newton rsqrt on vector only (no ACT table switch):
                # y0 = 1.09545 - 0.1895*v, then 4x y *= 1.5 - 0.5*v*y^2
                y = stat.tile([128, 1], F32, tag="y")
                nc.vector.tensor_scalar(y[:], v_e[:], -0.1895,
                                        1.09545, op0=ALU.mult, op1=ALU.add)
                for _ in range(4):
                    h2t = stat.tile([128, 1], F32, tag="h2t")
                    nc.vector.tensor_mul(h2t[:], y[:], y[:])
                    nc.vector.tensor_mul(h2t[:], h2t[:], v_e[:])
                    nc.vector.tensor_scalar(h2t[:], h2t[:], -0.5, 1.5,
                                            op0=ALU.mult, op1=ALU.add)
                    nc.vector.tensor_mul(y[:], y[:], h2t[:])
                nmr = stat.tile([128, 1], F32, tag="nmr")
                nc.vector.scalar_tensor_tensor(
                    nmr[:], mv[:, 0:1], -1.0, y[:],
                    op0=ALU.mult, op1=ALU.mult)
                for c in range(NOC):
                    slc = slice(c * OC, (c + 1) * OC)
                    yf = ep.tile([128, OC], F32, tag="yf")
                    nc.vector.tensor_scalar(
                        yf[:], r_full[:, slc], y[:], nmr[:],
                        op0=ALU.mult, op1=ALU.add)
                    if not cfg.trivial_affine:
                        y2 = ep.tile([128, OC], F32, tag="y2")
                        nc.vector.tensor_mul(y2[:], yf[:], gm_f[:, slc])
                        yf2 = ep.tile([128, OC], F32, tag="yf2")
                        nc.vector.tensor_add(yf2[:], y2[:], bt_f[:, slc])
                        yf = yf2
                    eng = nc.gpsimd if c % 2 == 0 else nc.scalar
                    eng.dma_start(
                        out=out[rt * 128:(rt + 1) * 128, slc],
                        in_=yf[:])

    nc.finalize()
    return nc


# ---------------------------------------------------------------------------
# host-side staging + sharding
# ---------------------------------------------------------------------------

def _bf16(a):
    import ml_dtypes
    return np.ascontiguousarray(np.asarray(a, np.float32)).astype(
        ml_dtypes.bfloat16)


def build_mask01(cfg: Cfg, j: int):
    # mk01[c, r] = 0 iff key (c = st*128 + k) is visible to the r-th query
    # of the diagonal block (c <= 64*j + r), else -1e9; same for every kb.
    c = np.arange(cfg.KB)[:, None]
    r = np.arange(cfg.g)[None, :]
    return np.where(c <= j * cfg.g + r, 0.0, -1.0e9).astype(np.float32)


def q_rows(cfg: Cfg, j: int):
    g = cfg.g
    return np.concatenate(
        [np.arange((j + 4 * i) * g, (j + 4 * i + 1) * g) for i in range(8)])


def make_in_map(cfg: Cfg, shared, xb_T_f32, xb_f32, j):
    rows = q_rows(cfg, j)
    return dict(
        shared,
        xtb=xb_T_f32,
        xtqb=np.ascontiguousarray(_bf16(xb_T_f32[:, rows])),
        xq=np.ascontiguousarray(xb_f32[rows]),
        mk01=build_mask01(cfg, j),
    )


def make_shared(cfg: Cfg, Wq, bq, Wk, bk, Wv, bv, Wo, bo, gamma, beta):
    H, KV, E, D = cfg.H, cfg.KV, cfg.E, cfg.D
    shared = {
        "wqb": _bf16(Wq),
        "wkb": np.ascontiguousarray(Wk, dtype=np.float32),
        "wvb": np.ascontiguousarray(Wv, dtype=np.float32),
        "wob": _bf16(Wo),
        "bqT": np.ascontiguousarray(
            np.asarray(bq, np.float32).reshape(H, E).T),
        "bkT": np.ascontiguousarray(
            np.asarray(bk, np.float32).reshape(KV, E).T),
        "bvb": np.ascontiguousarray(
            np.broadcast_to(np.asarray(bv, np.float32), (128, KV * E))),
    }
    if not cfg.trivial_affine:
        shared["bob"] = np.ascontiguousarray(
            np.broadcast_to(np.asarray(bo, np.float32), (128, D)))
        shared["gmb"] = np.ascontiguousarray(
            np.broadcast_to(np.asarray(gamma, np.float32), (128, D)))
        shared["btb"] = np.ascontiguousarray(
            np.broadcast_to(np.asarray(beta, np.float32), (128, D)))
    return shared


def assemble(cfg: Cfg, results, B):
    out = np.empty((B, cfg.L, cfg.D), np.float32)
    for core in range(4 * B):
        b, j = divmod(core, 4)
        out[b, q_rows(cfg, j)] = results[core]["out"]
    return out


_NC_CACHE = {}


def kernel(x, Wq, bq, Wk, bk, Wv, bv, Wo, bo, gamma, beta):
    from concourse.bass_utils import run_bass_kernel_spmd

    trivial = bool(
        np.all(np.asarray(gamma) == 1.0) and np.all(np.asarray(beta) == 0.0)
        and np.all(np.asarray(bo) == 0.0))
    cfg = Cfg(trivial_affine=trivial)
    if cfg not in _NC_CACHE:
        _NC_CACHE[cfg] = build_program(cfg)
    nc = _NC_CACHE[cfg]
    shared = make_shared(cfg, Wq, bq, Wk, bk, Wv, bv, Wo, bo, gamma, beta)
    x = np.asarray(x, np.float32)
    xT = [np.ascontiguousarray(x[b].T) for b in range(2)]
    in_maps = [make_in_map(cfg, shared, xT[core // 4], x[core // 4],
                           core % 4)
               for core in range(8)]
    res = run_bass_kernel_spmd(nc, in_maps, list(range(8)))
    return assemble(cfg, res.results, 2)


# revision 14
# speedup vs baseline: 1.0469x; 1.0469x over previous
"""Trainium2 Bass kernel for nn_Attention_Layer_78855599554595.

GQA attention layer: QKV proj -> causal GQA attention (16 heads, 4 kv heads,
E=128) -> out proj -> exact GELU -> residual -> LayerNorm.  B=2, L=2048, D=2048.

Sharding: zero-communication interleaved sequence parallelism.
  - 8 cores = 2 batches x 4 cores/batch.
  - Core j of a batch owns query rows in g=64-row blocks strided by 4:
    global blocks {j, j+4, ..., j+28} (512 rows).  For key block kb
    (256 keys), query-buffer columns [64*kb, 512) attend to it; the
    64-wide window at the start is the diagonal (host-supplied 0/1
    multiplicative mask, identical for every kb).
  - Each core computes K/V for its full batch (redundant 4x; on-chip
    collectives are slower than the 55us of PE time they would save).

v3 design (v2's trace showed the PE starving during the scalar-bound
attention phase, HAM-throttling to 1.2GHz; fixed by interleaving):
  - Q-projection of heads 8-15 is interleaved into heads 0-7's
    attention; out-proj partial sums over heads 0-7 are interleaved
    into heads 8-15's attention.  The PE never idles, stays at 2.4GHz,
    and half the out-projection rides under the exp shadow.
  - The causal mask is an additive -1e9 add on the vector engine on
    the PSUM scores (pre-exp), off the slow gpsimd path.

v2 design (from trace analysis of the 542us v1):
  - All matmuls run bf16 x bf16 (fp32 PSUM accumulate).  Host stages
    x^T and all weights as bf16: halves DMA, kills every fp32->fp32r
    tensor_copy, and removes the fp32r small-free-dim 4x penalty so the
    causal q-window never needs clamping.
  - K^T is produced directly by the projection matmul (Wk tile
    stationary, x^T moving) - no PE transposes, no scalar evictions.
  - K/V projection streams kt (contraction) innermost in 4 row-groups:
    first matmul needs only 1 weight tile + 1 x tile (~2us), not 8MB.
  - Softmax: exp in one scalar.activation per (head, key block) over
    both key subtiles ([128,2,qc] PSUM read); causal mask applied
    after exp as a 0/1 multiply on the gpsimd engine (SBUF-only);
    1/l broadcast via gpsimd.partition_broadcast (no PE broadcast
    matmul, no scalar copy).
  - Out-proj + GELU + residual + LN stats (vector bn_stats/bn_aggr)
    are fused per (oc, rt) tile; rstd via a vector-only Newton rsqrt
    (no scalar Sqrt -> no ACT table-set ping-pong); the LayerNorm +
    output DMA for each row tile drains right after its last column
    chunk, eliminating the 40us serial tail.

Host-side (free): transposes/gathers, bf16 staging, mask construction.
"""

import sys

sys.path.insert(0, "/opt/trn_rl_repo")

import numpy as np

from contextlib import ExitStack
from dataclasses import dataclass

from concourse import bacc, mybir, tile

F32 = mybir.dt.float32
R = mybir.dt.float32r
BF = mybir.dt.bfloat16
AF = mybir.ActivationFunctionType
ALU = mybir.AluOpType


@dataclass(frozen=True)
class Cfg:
    L: int = 2048          # sequence length (per batch)
    D: int = 2048          # model dim
    H: int = 16            # query heads
    KV: int = 4            # kv heads
    E: int = 128           # head dim (= partition width)
    trivial_affine: bool = False  # gamma==1, beta==0, bo==0: skip those ops
    act: object = None     # None -> exact GELU (CoreSim lacks Gelu; tests
                           # may override with an implemented fn like Tanh)

    @property
    def g(self):           # q block granularity (32 blocks across L)
        return self.L // 32

    @property
    def KB(self):          # key block size
        return self.L // 8

    @property
    def KSS(self):         # key subtile (partition) size
        return min(self.KB, 128)

    @property
    def ST(self):          # key subtiles per key block
        return max(1, self.KB // 128)

    @property
    def QR(self):          # query rows per core
        return self.L // 4

    @property
    def KT(self):          # contraction tiles over D
        return self.D // 128

    @property
    def RT(self):          # 128-row tiles of the core's q rows
        return self.QR // 128

    @property
    def OC(self):          # out-proj / LN column chunk
        return min(self.D, 512)


def build_program(cfg: Cfg):
    """Build the single-core SPMD Bass program. Returns finalized nc."""
    L, D, H, KV, E = cfg.L, cfg.D, cfg.H, cfg.KV, cfg.E
    g, KSS, ST, QR, KT, RT, OC = (cfg.g, cfg.KSS, cfg.ST, cfg.QR, cfg.KT,
                                  cfg.RT, cfg.OC)
    NOC = D // OC
    NKB = L // cfg.KB      # 8 key blocks
    inv_sqrt_e = 1.0 / float(np.sqrt(E))
    act_fn = cfg.act if cfg.act is not None else AF.Gelu

    nc = bacc.Bacc(None, target_bir_lowering=False)

    # ---- DRAM I/O (per-core data; same names on every core) ----
    xtb = nc.dram_tensor("xtb", [D, L], R, kind="ExternalInput")      # x[b].T
    xtqb = nc.dram_tensor("xtqb", [D, QR], BF, kind="ExternalInput")  # q cols
    xq = nc.dram_tensor("xq", [QR, D], F32, kind="ExternalInput")     # residual
    wqb = nc.dram_tensor("wqb", [D, H * E], BF, kind="ExternalInput")
    wkb = nc.dram_tensor("wkb", [D, KV * E], R, kind="ExternalInput")
    wvb = nc.dram_tensor("wvb", [D, KV * E], R, kind="ExternalInput")
    wob = nc.dram_tensor("wob", [H * E, D], BF, kind="ExternalInput")
    bqT = nc.dram_tensor("bqT", [E, H], F32, kind="ExternalInput")
    bkT = nc.dram_tensor("bkT", [E, KV], F32, kind="ExternalInput")
    bvb = nc.dram_tensor("bvb", [128, KV * E], F32, kind="ExternalInput")
    # additive diagonal mask (0 / -1e9) in S^T layout: [key (2x128), q (64)]
    mk01 = nc.dram_tensor("mk01", [cfg.KB, g], F32, kind="ExternalInput")
    if not cfg.trivial_affine:
        bob = nc.dram_tensor("bob", [128, D], F32, kind="ExternalInput")
        gmb = nc.dram_tensor("gmb", [128, D], F32, kind="ExternalInput")
        btb = nc.dram_tensor("btb", [128, D], F32, kind="ExternalInput")
    out = nc.dram_tensor("out", [QR, D], F32, kind="ExternalOutput")

    with tile.TileContext(nc) as tc, ExitStack() as top:
        const = top.enter_context(tc.tile_pool(name="const", bufs=1))
        ctxp = top.enter_context(tc.tile_pool(name="ctxp", bufs=1))
        pap = top.enter_context(tc.tile_pool(name="pap", bufs=1))
        kvq_stack = ExitStack()
        kvq = kvq_stack.enter_context(tc.tile_pool(name="kvq", bufs=1))
        qtp = kvq_stack.enter_context(tc.tile_pool(name="qtp", bufs=1))

        # ---- constants (vector DMA queue; issued at t=0) ----
        ones_kb = const.tile([128, 2], BF)
        nc.gpsimd.memset(ones_kb[:], 1.0)
        bq_t = const.tile([E, H], F32)
        bk_t = const.tile([E, KV], F32)
        bv_t = const.tile([128, KV * E], F32)
        mk_t = const.tile([KSS, ST, g], F32)
        nc.gpsimd.dma_start(out=bq_t[:], in_=bqT[:])
        nc.gpsimd.dma_start(out=bk_t[:], in_=bkT[:])
        nc.gpsimd.dma_start(out=bv_t[:], in_=bvb[:])
        nc.gpsimd.dma_start(out=mk_t[:],
                            in_=mk01.rearrange("(s p) q -> p s q", p=KSS))

        # persistent activations
        kT = [kvq.tile([E, L], BF, tag=f"kT{kv}", name=f"kT{kv}")
              for kv in range(KV)]
        vN = kvq.tile([KSS, L // KSS, KV * E], BF)      # V natural, key-major
        qT = [qtp.tile([E, QR], BF, tag=f"qT{h}", name=f"qT{h}")
              for h in range(H)]
        ctxT = [ctxp.tile([E, QR], BF, tag=f"cT{h}", name=f"cT{h}")
                for h in range(H)]
        # out-proj partial sums over heads 0..7 (filled during phase 3b)
        pA = [pap.tile([128, D], F32, tag=f"pA{rt}", name=f"pA{rt}")
              for rt in range(RT)]

        # phase-2 moving operand: resident x^T at q rows (vector queue, t=0)
        xtq_stack = ExitStack()
        xtqp = xtq_stack.enter_context(tc.tile_pool(name="xtqp", bufs=1))
        xtq_r = xtqp.tile([128, KT, QR], BF)

        def emit_xtq_loads():   # called after phase-1 group 0
            for kt in range(KT):
                nc.scalar.dma_start(out=xtq_r[:, kt, :],
                                    in_=xtqb[kt * 128:(kt + 1) * 128, :])

        # ================= Phase 1: K/V projections ========================
        # kT direct:  out[E, rows] += Wk_tile.T @ xT_tile   (moving = x^T)
        # V natural:  out[rows, kvE] += xT_tile.T @ Wv_tile (moving = Wv)
        with ExitStack() as ph:
            wkv = ph.enter_context(tc.tile_pool(name="wkv", bufs=1))
            stage = ph.enter_context(tc.tile_pool(name="stage1", bufs=4))
            ps1 = ph.enter_context(tc.tile_pool(name="ps1", bufs=1,
                                                space="PSUM"))
            wk_r = wkv.tile([128, KT, KV * E], R)
            wv_r = wkv.tile([128, KT, KV * E], R)
            for kt in range(KT):
                nc.sync.dma_start(out=wk_r[:, kt, :],
                                  in_=wkb[kt * 128:(kt + 1) * 128, :])
                nc.sync.dma_start(out=wv_r[:, kt, :],
                                  in_=wvb[kt * 128:(kt + 1) * 128, :])

            NG = L // 512
            for g4 in range(NG):
                c0 = 512 * g4
                pKT = [ps1.tile([E, 512], F32, tag=f"pKT{kv}",
                                name=f"pKT{kv}") for kv in range(KV)]
                pV = [ps1.tile([128, KV * E], F32, tag=f"pV{c}",
                               name=f"pV{c}") for c in range(4)]
                for kt in range(KT):
                    xg = stage.tile([128, 512], R, tag="xg")
                    eng = nc.gpsimd if kt % 2 == 0 else nc.scalar
                    eng.dma_start(
                        out=xg[:], in_=xtb[kt * 128:(kt + 1) * 128,
                                           c0:c0 + 512])
                    for kv in range(KV):
                        nc.tensor.matmul(
                            pKT[kv][:], wk_r[:, kt, kv * E:(kv + 1) * E],
                            xg[:], start=(kt == 0), stop=(kt == KT - 1))
                    for c in range(4):
                        nc.tensor.matmul(
                            pV[c][:], xg[:, c * 128:(c + 1) * 128],
                            wv_r[:, kt, :], start=(kt == 0),
                            stop=(kt == KT - 1))
                for kv in range(KV):
                    nc.vector.tensor_scalar(
                        kT[kv][:, c0:c0 + 512], pKT[kv][:],
                        bk_t[:, kv:kv + 1], None, op0=ALU.add)
                for c in range(4):
                    nc.vector.tensor_add(vN[:, g4 * 4 + c, :], pV[c][:],
                                         bv_t[:])
                if g4 == 0:
                    emit_xtq_loads()

        # ====== Phase 2a: Q^T projection, heads 0..7 =======================
        # (heads 8..15 are projected inside phase 3a as PE filler work)
        with ExitStack() as ph:
            stage = ph.enter_context(tc.tile_pool(name="stage2", bufs=8))
            ps2 = ph.enter_context(tc.tile_pool(name="ps2", bufs=1,
                                                space="PSUM"))
            HB = 8
            pqs = [ps2.tile([E, QR], F32, tag=f"pq{hh}", name=f"pq{hh}")
                   for hh in range(HB)]
            for kt in range(KT):
                wqs = stage.tile([128, HB, E], BF, tag="wqs")
                nc.sync.dma_start(
                    out=wqs[:],
                    in_=wqb[kt * 128:(kt + 1) * 128, 0:HB * E]
                    .rearrange("p (h e) -> p h e", h=HB))
                for hh in range(HB):
                    nc.tensor.matmul(
                        pqs[hh][:], wqs[:, hh, :], xtq_r[:, kt, :],
                        start=(kt == 0), stop=(kt == KT - 1))
            for hh in range(HB):
                nc.vector.tensor_scalar(
                    qT[hh][:], pqs[hh][:], bq_t[:, hh:hh + 1], None,
                    op0=ALU.add)

        # ================= Phase 3: attention ==============================
        # The exp stream on the scalar engine is the phase bottleneck, so
        # the PE is kept busy (and HAM-warm) with interleaved filler:
        #   heads 0..7:  Q-projection of head 8+h (2 matmuls / kb slot)
        #   heads 8..15: out-proj partial sums over heads 0..7 into pA
        with ExitStack() as ph:
            ps_s = ph.enter_context(tc.tile_pool(name="pss", bufs=2,
                                                 space="PSUM"))
            ps_c = ph.enter_context(tc.tile_pool(name="psc", bufs=2,
                                                 space="PSUM"))
            ps_l = ph.enter_context(tc.tile_pool(name="psl", bufs=1,
                                                 space="PSUM"))
            ps_f = ph.enter_context(tc.tile_pool(name="psf", bufs=1,
                                                 space="PSUM"))
            exp_p = ph.enter_context(tc.tile_pool(name="expp", bufs=3))
            lso = ph.enter_context(tc.tile_pool(name="lso", bufs=2))
            stage3 = ph.enter_context(tc.tile_pool(name="stage3", bufs=2))
            woAp = ph.enter_context(tc.tile_pool(name="woAp", bufs=1))

            # wo rows for heads 0..7 (the 3b filler's moving operand)
            woA = woAp.tile([128, H // 2, D], BF)
            for hh in range(H // 2):
                nc.sync.dma_start(out=woA[:, hh, :],
                                  in_=wob[hh * E:(hh + 1) * E, :])

            wq2 = {}

            def emit_wq2_dma(h2):
                if h2 >= H:
                    return
                w = stage3.tile([128, KT, E], BF, tag="wq2",
                                name=f"wq2_{h2}")
                nc.sync.dma_start(
                    out=w[:],
                    in_=wqb[:, h2 * E:(h2 + 1) * E]
                    .rearrange("(kt p) e -> p kt e", p=128))
                wq2[h2] = w

            emit_wq2_dma(8)
            fill_state = {"pq2": None, "pys": None, "kt": 0, "gs": 0}
            # filler matmuls per kb slot, weighted toward the small-qc
            # (PE-light) late key blocks so exp latency stays hidden
            FILL_W = [1, 1, 1, 2, 2, 2, 3, 4]
            assert sum(FILL_W) == 16

            def emit_filler(h, kb):
                n = FILL_W[kb]
                if h < H // 2:
                    h2 = 8 + h
                    if kb == 0:
                        fill_state["pq2"] = ps_f.tile([E, QR], F32,
                                                      tag="fill",
                                                      name=f"pq2_{h2}")
                        fill_state["kt"] = 0
                    pq2 = fill_state["pq2"]
                    for _ in range(n):
                        kt = fill_state["kt"]
                        fill_state["kt"] += 1
                        nc.tensor.matmul(
                            pq2[:], wq2[h2][:, kt, :], xtq_r[:, kt, :],
                            start=(kt == 0), stop=(kt == KT - 1))
                    if kb == NKB - 1:
                        nc.vector.tensor_scalar(
                            qT[h2][:], pq2[:], bq_t[:, h2:h2 + 1], None,
                            op0=ALU.add)
                else:
                    for _ in range(n):
                        gs = fill_state["gs"]
                        fill_state["gs"] += 1
                        u, step = divmod(gs, 8)
                        oc, rt = divmod(u, RT)
                        if step == 0:
                            fill_state["pys"] = ps_f.tile([128, OC], F32,
                                                          tag="fill",
                                                          name=f"pysA_{u}")
                        pys = fill_state["pys"]
                        nc.tensor.matmul(
                            pys[:], ctxT[step][:, rt * 128:(rt + 1) * 128],
                            woA[:, step, oc * OC:(oc + 1) * OC],
                            start=(step == 0), stop=(step == 7))
                        if step == 7:
                            nc.vector.tensor_copy(
                                pA[rt][:, oc * OC:(oc + 1) * OC], pys[:])

            for h in range(H):
                kv = h % KV
                if h < H // 2:
                    emit_wq2_dma(9 + h)
                pctx = ps_c.tile([E, QR], F32, tag="pctx")
                pl = ps_l.tile([2, QR], F32, tag="pl")
                eSs = [None] * NKB
                q0s = [None] * NKB

                def emit_pl_ctx(kb):
                    eS, q0 = eSs[kb], q0s[kb]
                    qc = QR - q0
                    first = (kb == 0)
                    last = (kb == NKB - 1)
                    for st in range(ST):
                        k0 = (kb * ST + st)
                        nc.tensor.matmul(
                            pl[:, q0:], ones_kb[:, :], eS[:, st, :qc],
                            start=first and st == 0,
                            stop=last and st == ST - 1,
                            skip_group_check=True)
                        nc.tensor.matmul(
                            pctx[:, q0:], vN[:, k0, kv * E:(kv + 1) * E],
                            eS[:, st, :qc],
                            start=first and st == 0,
                            stop=last and st == ST - 1,
                            skip_group_check=True)

                for kb in range(NKB):
                    q0 = g * kb
                    qc = QR - q0
                    q0s[kb] = q0
                    pS = ps_s.tile([KSS, ST, QR], F32, tag="pS")
                    for st in range(ST):
                        k0 = kb * cfg.KB + st * KSS
                        nc.tensor.matmul(pS[:, st, :qc],
                                         kT[kv][:, k0:k0 + KSS],
                                         qT[h][:, q0:], start=True, stop=True)
                    emit_filler(h, kb)
                    # pipeline: previous block's pl/pctx go behind these
                    # scores so the PE isn't blocked on this block's exp.
                    if kb > 0:
                        emit_pl_ctx(kb - 1)
                    # additive causal mask (0/-1e9) on the diagonal window
                    nc.vector.tensor_add(pS[:, :, :g], pS[:, :, :g], mk_t[:])
                    eS = exp_p.tile([KSS, ST, QR], BF, tag="eS")
                    nc.scalar.activation(eS[:, :, :qc], pS[:, :, :qc],
                                         AF.Exp, scale=inv_sqrt_e)
                    eSs[kb] = eS
                emit_pl_ctx(NKB - 1)

                l2f = lso.tile([1, QR], F32, tag="l2f")
                nc.vector.reciprocal_approx_fast(l2f[:], pl[:1, :])
                rb = lso.tile([128, QR], F32, tag="rb")
                nc.gpsimd.partition_broadcast(rb[:], l2f[:])
                nc.vector.tensor_mul(ctxT[h][:], pctx[:], rb[:])

        xtq_stack.close()
        kvq_stack.close()

        # ===== Phase 4+5: out-proj (heads 8..15) + GELU + residual + LN ====
        # rt-outer with wo fully resident: each row tile's LayerNorm +
        # output DMA drains while the next row tile's matmuls run.
        with ExitStack() as ph:
            wop = ph.enter_context(tc.tile_pool(name="wop", bufs=1))
            rfp = ph.enter_context(tc.tile_pool(name="rfp", bufs=2))
            ps_y = ph.enter_context(tc.tile_pool(name="psy", bufs=4,
                                                 space="PSUM"))
            ep = ph.enter_context(tc.tile_pool(name="epp", bufs=3))
            stat = ph.enter_context(tc.tile_pool(name="stat", bufs=1))
            gbp = ph.enter_context(tc.tile_pool(name="gbp", bufs=1))

            woB = wop.tile([128, H // 2, D], BF)
            for hh in range(H // 2):
                h2 = H // 2 + hh
                nc.sync.dma_start(out=woB[:, hh, :],
                                  in_=wob[h2 * E:(h2 + 1) * E, :])
            if not cfg.trivial_affine:
                bo_f = gbp.tile([128, D], F32)
                gm_f = gbp.tile([128, D], F32)
                bt_f = gbp.tile([128, D], F32)
                nc.scalar.dma_start(out=bo_f[:], in_=bob[:])
                nc.scalar.dma_start(out=gm_f[:], in_=gmb[:])
                nc.scalar.dma_start(out=bt_f[:], in_=btb[:])

            for rt in range(RT):
                r_full = rfp.tile([128, D], F32, tag="rf")
                bna = stat.tile([128, NOC, 6], F32, tag="bna")
                for oc in range(NOC):
                    pys = ps_y.tile([128, OC], F32, tag="pys")
                    for hh in range(H // 2):
                        nc.tensor.matmul(
                            pys[:],
                            ctxT[H // 2 + hh][:, rt * 128:(rt + 1) * 128],
                            woB[:, hh, oc * OC:(oc + 1) * OC],
                            start=(hh == 0), stop=(hh == H // 2 - 1))
                    tb = ep.tile([128, OC], F32, tag="tb")
                    nc.vector.tensor_add(tb[:], pys[:],
                                         pA[rt][:, oc * OC:(oc + 1) * OC])
                    if not cfg.trivial_affine:
                        tb2 = ep.tile([128, OC], F32, tag="tb2")
                        nc.vector.tensor_add(
                            tb2[:], tb[:], bo_f[:, oc * OC:(oc + 1) * OC])
                        tb = tb2
                    t2 = ep.tile([128, OC], F32, tag="t2")
                    nc.scalar.activation(t2[:], tb[:], act_fn)
                    xqt = ep.tile([128, OC], F32, tag="xqt")
                    nc.scalar.dma_start(
                        out=xqt[:],
                        in_=xq[rt * 128:(rt + 1) * 128, oc * OC:(oc + 1) * OC])
                    rch = r_full[:, oc * OC:(oc + 1) * OC]
                    nc.vector.tensor_add(rch, t2[:], xqt[:])
                    nc.vector.bn_stats(bna[:, oc, :], rch)
                # stats complete for this row tile: LN + drain now, while
                # the next row tile's matmuls occupy the PE.
                mv = stat.tile([128, 2], F32, tag="mv")
                nc.vector.bn_aggr(mv[:], bna[:])
                v_e = stat.tile([128, 1], F32, tag="ve")
                nc.vector.tensor_scalar_add(v_e[:], mv[:, 1:2], 1e-5)
                # Newton rsqrt on vector only (no ACT table switch):
                # y0 = 1.09545 - 0.1895*v, then 4x y *= 1.5 - 0.5*v*y^2
                y = stat.tile([128, 1], F32, tag="y")
                nc.vector.tensor_scalar(y[:], v_e[:], -0.1895,
                                        1.09545, op0=ALU.mult, op1=ALU.add)
                for _ in range(4):
                    h2t = stat.tile([128, 1], F32, tag="h2t")
                    nc.vector.tensor_mul(h2t[:], y[:], y[:])
                    nc.vector.tensor_mul(h2t[:], h2t[:], v_e[:])
                    nc.vector.tensor_scalar(h2t[:], h2t[:], -0.5, 1.5,
                                            op0=ALU.mult, op1=ALU.add)
                    nc.vector.tensor_mul(y[:], y[:], h2t[:])
                nmr = stat.tile([128, 1], F32, tag="nmr")
                nc.vector.scalar_tensor_tensor(
                    nmr[:], mv[:, 0:1], -1.0, y[:],
                    op0=ALU.mult, op1=ALU.mult)
                for c in range(NOC):
                    slc = slice(c * OC, (c + 1) * OC)
                    yf = ep.tile([128, OC], F32, tag="yf")
                    nc.vector.tensor_scalar(
                        yf[:], r_full[:, slc], y[:], nmr[:],
                        op0=ALU.mult, op1=ALU.add)
                    if not cfg.trivial_affine:
                        y2 = ep.tile([128, OC], F32, tag="y2")
                        nc.vector.tensor_mul(y2[:], yf[:], gm_f[:, slc])
                        yf2 = ep.tile([128, OC], F32, tag="yf2")
                        nc.vector.tensor_add(yf2[:], y2[:], bt_f[:, slc])
                        yf = yf2
                    eng = (nc.gpsimd, nc.scalar,
                           nc.sync)[(rt * NOC + c) % 3]
                    eng.dma_start(
                        out=out[rt * 128:(rt + 1) * 128, slc],
                        in_=yf[:])

    nc.finalize()
    return nc


# ---------------------------------------------------------------------------
# host-side staging + sharding
# ---------------------------------------------------------------------------

def _bf16(a):
    import ml_dtypes
    return np.ascontiguousarray(np.asarray(a, np.float32)).astype(
        ml_dtypes.bfloat16)


def build_mask01(cfg: Cfg, j: int):
    # mk01[c, r] = 0 iff key (c = st*128 + k) is visible to the r-th query
    # of the diagonal block (c <= 64*j + r), else -1e9; same for every kb.
    c = np.arange(cfg.KB)[:, None]
    r = np.arange(cfg.g)[None, :]
    return np.where(c <= j * cfg.g + r, 0.0, -1.0e9).astype(np.float32)


def q_rows(cfg: Cfg, j: int):
    g = cfg.g
    return np.concatenate(
        [np.arange((j + 4 * i) * g, (j + 4 * i + 1) * g) for i in range(8)])


def make_in_map(cfg: Cfg, shared, xb_T_f32, xb_f32, j):
    rows = q_rows(cfg, j)
    return dict(
        shared,
        xtb=xb_T_f32,
        xtqb=np.ascontiguousarray(_bf16(xb_T_f32[:, rows])),
        xq=np.ascontiguousarray(xb_f32[rows]),
        mk01=build_mask01(cfg, j),
    )


def make_shared(cfg: Cfg, Wq, bq, Wk, bk, Wv, bv, Wo, bo, gamma, beta):
    H, KV, E, D = cfg.H, cfg.KV, cfg.E, cfg.D
    shared = {
        "wqb": _bf16(Wq),
        "wkb": np.ascontiguousarray(Wk, dtype=np.float32),
        "wvb": np.ascontiguousarray(Wv, dtype=np.float32),
        "wob": _bf16(Wo),
        "bqT": np.ascontiguousarray(
            np.asarray(bq, np.float32).reshape(H, E).T),
        "bkT": np.ascontiguousarray(
            np.asarray(bk, np.float32).reshape(KV, E).T),
        "bvb": np.ascontiguousarray(
            np.broadcast_to(np.asarray(bv, np.float32), (128, KV * E))),
    }
    if not cfg.trivial_affine:
        shared["bob"] = np.ascontiguousarray(
            np.broadcast_to(np.asarray(bo, np.float32), (128, D)))
        shared["gmb"] = np.ascontiguousarray(
            np.broadcast_to(np.asarray(gamma, np.float32), (128, D)))
        shared["btb"] = np.ascontiguousarray(
            np.broadcast_to(np.asarray(beta, np.float32), (128, D)))
    return shared


def assemble(cfg: Cfg, results, B):
    out = np.empty((B, cfg.L, cfg.D), np.float32)
    for core in range(4 * B):
        b, j = divmod(core, 4)
        out[b, q_rows(cfg, j)] = results[core]["out"]
    return out


_NC_CACHE = {}


def kernel(x, Wq, bq, Wk, bk, Wv, bv, Wo, bo, gamma, beta):
    from concourse.bass_utils import run_bass_kernel_spmd

    trivial = bool(
        np.all(np.asarray(gamma) == 1.0) and np.all(np.asarray(beta) == 0.0)
        and np.all(np.asarray(bo) == 0.0))
    cfg = Cfg(trivial_affine=trivial)
    if cfg not in _NC_CACHE:
        _NC_CACHE[cfg] = build_program(cfg)
    nc = _NC_CACHE[cfg]
    shared = make_shared(cfg, Wq, bq, Wk, bk, Wv, bv, Wo, bo, gamma, beta)
    x = np.asarray(x, np.float32)
    xT = [np.ascontiguousarray(x[b].T) for b in range(2)]
    in_maps = [make_in_map(cfg, shared, xT[core // 4], x[core // 4],
                           core % 4)
               for core in range(8)]
    res = run_bass_kernel_spmd(nc, in_maps, list(range(8)))
    return assemble(cfg, res.results, 2)


# revision 15
# speedup vs baseline: 1.0618x; 1.0142x over previous
"""Trainium2 Bass kernel for nn_Attention_Layer_78855599554595.

GQA attention layer: QKV proj -> causal GQA attention (16 heads, 4 kv heads,
E=128) -> out proj -> exact GELU -> residual -> LayerNorm.  B=2, L=2048, D=2048.

Sharding: zero-communication interleaved sequence parallelism.
  - 8 cores = 2 batches x 4 cores/batch.
  - Core j of a batch owns query rows in g=64-row blocks strided by 4:
    global blocks {j, j+4, ..., j+28} (512 rows).  For key block kb
    (256 keys), query-buffer columns [64*kb, 512) attend to it; the
    64-wide window at the start is the diagonal (host-supplied 0/1
    multiplicative mask, identical for every kb).
  - Each core computes K/V for its full batch (redundant 4x; on-chip
    collectives are slower than the 55us of PE time they would save).

v3 design (v2's trace showed the PE starving during the scalar-bound
attention phase, HAM-throttling to 1.2GHz; fixed by interleaving):
  - Q-projection of heads 8-15 is interleaved into heads 0-7's
    attention; out-proj partial sums over heads 0-7 are interleaved
    into heads 8-15's attention.  The PE never idles, stays at 2.4GHz,
    and half the out-projection rides under the exp shadow.
  - The causal mask is an additive -1e9 add on the vector engine on
    the PSUM scores (pre-exp), off the slow gpsimd path.

v2 design (from trace analysis of the 542us v1):
  - All matmuls run bf16 x bf16 (fp32 PSUM accumulate).  Host stages
    x^T and all weights as bf16: halves DMA, kills every fp32->fp32r
    tensor_copy, and removes the fp32r small-free-dim 4x penalty so the
    causal q-window never needs clamping.
  - K^T is produced directly by the projection matmul (Wk tile
    stationary, x^T moving) - no PE transposes, no scalar evictions.
  - K/V projection streams kt (contraction) innermost in 4 row-groups:
    first matmul needs only 1 weight tile + 1 x tile (~2us), not 8MB.
  - Softmax: exp in one scalar.activation per (head, key block) over
    both key subtiles ([128,2,qc] PSUM read); causal mask applied
    after exp as a 0/1 multiply on the gpsimd engine (SBUF-only);
    1/l broadcast via gpsimd.partition_broadcast (no PE broadcast
    matmul, no scalar copy).
  - Out-proj + GELU + residual + LN stats (vector bn_stats/bn_aggr)
    are fused per (oc, rt) tile; rstd via a vector-only Newton rsqrt
    (no scalar Sqrt -> no ACT table-set ping-pong); the LayerNorm +
    output DMA for each row tile drains right after its last column
    chunk, eliminating the 40us serial tail.

Host-side (free): transposes/gathers, bf16 staging, mask construction.
"""

import sys

sys.path.insert(0, "/opt/trn_rl_repo")

import numpy as np

from contextlib import ExitStack
from dataclasses import dataclass

from concourse import bacc, mybir, tile

F32 = mybir.dt.float32
R = mybir.dt.float32r
BF = mybir.dt.bfloat16
AF = mybir.ActivationFunctionType
ALU = mybir.AluOpType


@dataclass(frozen=True)
class Cfg:
    L: int = 2048          # sequence length (per batch)
    D: int = 2048          # model dim
    H: int = 16            # query heads
    KV: int = 4            # kv heads
    E: int = 128           # head dim (= partition width)
    trivial_affine: bool = False  # gamma==1, beta==0, bo==0: skip those ops
    act: object = None     # None -> exact GELU (CoreSim lacks Gelu; tests
                           # may override with an implemented fn like Tanh)

    @property
    def g(self):           # q block granularity (32 blocks across L)
        return self.L // 32

    @property
    def KB(self):          # key block size
        return self.L // 8

    @property
    def KSS(self):         # key subtile (partition) size
        return min(self.KB, 128)

    @property
    def ST(self):          # key subtiles per key block
        return max(1, self.KB // 128)

    @property
    def QR(self):          # query rows per core
        return self.L // 4

    @property
    def KT(self):          # contraction tiles over D
        return self.D // 128

    @property
    def RT(self):          # 128-row tiles of the core's q rows
        return self.QR // 128

    @property
    def OC(self):          # out-proj / LN column chunk
        return min(self.D, 512)


def build_program(cfg: Cfg):
    """Build the single-core SPMD Bass program. Returns finalized nc."""
    L, D, H, KV, E = cfg.L, cfg.D, cfg.H, cfg.KV, cfg.E
    g, KSS, ST, QR, KT, RT, OC = (cfg.g, cfg.KSS, cfg.ST, cfg.QR, cfg.KT,
                                  cfg.RT, cfg.OC)
    NOC = D // OC
    NKB = L // cfg.KB      # 8 key blocks
    inv_sqrt_e = 1.0 / float(np.sqrt(E))
    act_fn = cfg.act if cfg.act is not None else AF.Gelu

    nc = bacc.Bacc(None, target_bir_lowering=False)

    # ---- DRAM I/O (per-core data; same names on every core) ----
    xtb = nc.dram_tensor("xtb", [D, L], R, kind="ExternalInput")      # x[b].T
    xtqb = nc.dram_tensor("xtqb", [D, QR], BF, kind="ExternalInput")  # q cols
    xq = nc.dram_tensor("xq", [QR, D], F32, kind="ExternalInput")     # residual
    wqb = nc.dram_tensor("wqb", [D, H * E], BF, kind="ExternalInput")
    wkb = nc.dram_tensor("wkb", [D, KV * E], R, kind="ExternalInput")
    wvb = nc.dram_tensor("wvb", [D, KV * E], R, kind="ExternalInput")
    wob = nc.dram_tensor("wob", [H * E, D], BF, kind="ExternalInput")
    bqT = nc.dram_tensor("bqT", [E, H], F32, kind="ExternalInput")
    bkT = nc.dram_tensor("bkT", [E, KV], F32, kind="ExternalInput")
    bvb = nc.dram_tensor("bvb", [128, KV * E], F32, kind="ExternalInput")
    # additive diagonal mask (0 / -1e9) in S^T layout: [key (2x128), q (64)]
    mk01 = nc.dram_tensor("mk01", [cfg.KB, g], F32, kind="ExternalInput")
    if not cfg.trivial_affine:
        bob = nc.dram_tensor("bob", [128, D], F32, kind="ExternalInput")
        gmb = nc.dram_tensor("gmb", [128, D], F32, kind="ExternalInput")
        btb = nc.dram_tensor("btb", [128, D], F32, kind="ExternalInput")
    out = nc.dram_tensor("out", [QR, D], F32, kind="ExternalOutput")

    with tile.TileContext(nc) as tc, ExitStack() as top:
        const = top.enter_context(tc.tile_pool(name="const", bufs=1))
        ctxp = top.enter_context(tc.tile_pool(name="ctxp", bufs=1))
        pap = top.enter_context(tc.tile_pool(name="pap", bufs=1))
        kvq_stack = ExitStack()
        kvq = kvq_stack.enter_context(tc.tile_pool(name="kvq", bufs=1))
        qtp = kvq_stack.enter_context(tc.tile_pool(name="qtp", bufs=1))

        # ---- constants (vector DMA queue; issued at t=0) ----
        ones_kb = const.tile([128, 2], BF)
        nc.gpsimd.memset(ones_kb[:], 1.0)
        bq_t = const.tile([E, H], F32)
        bk_t = const.tile([E, KV], F32)
        bv_t = const.tile([128, KV * E], F32)
        mk_t = const.tile([KSS, ST, g], F32)
        nc.gpsimd.dma_start(out=bq_t[:], in_=bqT[:])
        nc.gpsimd.dma_start(out=bk_t[:], in_=bkT[:])
        nc.gpsimd.dma_start(out=bv_t[:], in_=bvb[:])
        nc.gpsimd.dma_start(out=mk_t[:],
                            in_=mk01.rearrange("(s p) q -> p s q", p=KSS))

        # persistent activations
        kT = [kvq.tile([E, L], BF, tag=f"kT{kv}", name=f"kT{kv}")
              for kv in range(KV)]
        vN = kvq.tile([KSS, L // KSS, KV * E], BF)      # V natural, key-major
        qT = [qtp.tile([E, QR], BF, tag=f"qT{h}", name=f"qT{h}")
              for h in range(H)]
        ctxT = [ctxp.tile([E, QR], BF, tag=f"cT{h}", name=f"cT{h}")
                for h in range(H)]
        # out-proj partial sums over heads 0..7 (filled during phase 3b)
        pA = [pap.tile([128, D], F32, tag=f"pA{rt}", name=f"pA{rt}")
              for rt in range(RT)]

        # phase-2 moving operand: resident x^T at q rows (vector queue, t=0)
        xtq_stack = ExitStack()
        xtqp = xtq_stack.enter_context(tc.tile_pool(name="xtqp", bufs=1))
        xtq_r = xtqp.tile([128, KT, QR], BF)

        def emit_xtq_loads():   # called after phase-1 group 0
            for kt in range(KT):
                nc.scalar.dma_start(out=xtq_r[:, kt, :],
                                    in_=xtqb[kt * 128:(kt + 1) * 128, :])

        # ---- PE warm-up: ~6us of dependency-free matmuls so HAM is at
        # full clock by the time the first weights/x tiles arrive ----
        with ExitStack() as ph:
            wrm_p = ph.enter_context(tc.tile_pool(name="wrmp", bufs=1))
            ps_w = ph.enter_context(tc.tile_pool(name="psw", bufs=1,
                                                 space="PSUM"))
            wrm = wrm_p.tile([128, 512], BF)
            nc.gpsimd.memset(wrm[:], 0.0)
            pwrm = ps_w.tile([2, 512], F32)
            for i in range(16):
                nc.tensor.matmul(pwrm[:], ones_kb[:, :], wrm[:],
                                 start=(i == 0), stop=(i == 15))

        # ================= Phase 1: K/V projections ========================
        # kT direct:  out[E, rows] += Wk_tile.T @ xT_tile   (moving = x^T)
        # V natural:  out[rows, kvE] += xT_tile.T @ Wv_tile (moving = Wv)
        with ExitStack() as ph:
            wkv = ph.enter_context(tc.tile_pool(name="wkv", bufs=1))
            stage = ph.enter_context(tc.tile_pool(name="stage1", bufs=4))
            ps1 = ph.enter_context(tc.tile_pool(name="ps1", bufs=1,
                                                space="PSUM"))
            wk_r = wkv.tile([128, KT, KV * E], R)
            wv_r = wkv.tile([128, KT, KV * E], R)
            for kt in range(KT):
                nc.sync.dma_start(out=wk_r[:, kt, :],
                                  in_=wkb[kt * 128:(kt + 1) * 128, :])
                nc.sync.dma_start(out=wv_r[:, kt, :],
                                  in_=wvb[kt * 128:(kt + 1) * 128, :])

            NG = L // 512
            for g4 in range(NG):
                c0 = 512 * g4
                pKT = [ps1.tile([E, 512], F32, tag=f"pKT{kv}",
                                name=f"pKT{kv}") for kv in range(KV)]
                pV = [ps1.tile([128, KV * E], F32, tag=f"pV{c}",
                               name=f"pV{c}") for c in range(4)]
                for kt in range(KT):
                    xg = stage.tile([128, 512], R, tag="xg")
                    eng = nc.gpsimd if kt % 2 == 0 else nc.scalar
                    eng.dma_start(
                        out=xg[:], in_=xtb[kt * 128:(kt + 1) * 128,
                                           c0:c0 + 512])
                    for kv in range(KV):
                        nc.tensor.matmul(
                            pKT[kv][:], wk_r[:, kt, kv * E:(kv + 1) * E],
                            xg[:], start=(kt == 0), stop=(kt == KT - 1))
                    for c in range(4):
                        nc.tensor.matmul(
                            pV[c][:], xg[:, c * 128:(c + 1) * 128],
                            wv_r[:, kt, :], start=(kt == 0),
                            stop=(kt == KT - 1))
                for kv in range(KV):
                    nc.vector.tensor_scalar(
                        kT[kv][:, c0:c0 + 512], pKT[kv][:],
                        bk_t[:, kv:kv + 1], None, op0=ALU.add)
                for c in range(4):
                    nc.vector.tensor_add(vN[:, g4 * 4 + c, :], pV[c][:],
                                         bv_t[:])
                if g4 == 0:
                    emit_xtq_loads()

        # ====== Phase 2a: Q^T projection, heads 0..7 =======================
        # (heads 8..15 are projected inside phase 3a as PE filler work)
        with ExitStack() as ph:
            stage = ph.enter_context(tc.tile_pool(name="stage2", bufs=8))
            ps2 = ph.enter_context(tc.tile_pool(name="ps2", bufs=1,
                                                space="PSUM"))
            HB = 8
            pqs = [ps2.tile([E, QR], F32, tag=f"pq{hh}", name=f"pq{hh}")
                   for hh in range(HB)]
            for kt in range(KT):
                wqs = stage.tile([128, HB, E], BF, tag="wqs")
                nc.sync.dma_start(
                    out=wqs[:],
                    in_=wqb[kt * 128:(kt + 1) * 128, 0:HB * E]
                    .rearrange("p (h e) -> p h e", h=HB))
                for hh in range(HB):
                    nc.tensor.matmul(
                        pqs[hh][:], wqs[:, hh, :], xtq_r[:, kt, :],
                        start=(kt == 0), stop=(kt == KT - 1))
            for hh in range(HB):
                nc.vector.tensor_scalar(
                    qT[hh][:], pqs[hh][:], bq_t[:, hh:hh + 1], None,
                    op0=ALU.add)

        # ================= Phase 3: attention ==============================
        # The exp stream on the scalar engine is the phase bottleneck, so
        # the PE is kept busy (and HAM-warm) with interleaved filler:
        #   heads 0..7:  Q-projection of head 8+h (2 matmuls / kb slot)
        #   heads 8..15: out-proj partial sums over heads 0..7 into pA
        with ExitStack() as ph:
            ps_s = ph.enter_context(tc.tile_pool(name="pss", bufs=2,
                                                 space="PSUM"))
            ps_c = ph.enter_context(tc.tile_pool(name="psc", bufs=2,
                                                 space="PSUM"))
            ps_l = ph.enter_context(tc.tile_pool(name="psl", bufs=1,
                                                 space="PSUM"))
            ps_f = ph.enter_context(tc.tile_pool(name="psf", bufs=1,
                                                 space="PSUM"))
            exp_p = ph.enter_context(tc.tile_pool(name="expp", bufs=3))
            lso = ph.enter_context(tc.tile_pool(name="lso", bufs=2))
            stage3 = ph.enter_context(tc.tile_pool(name="stage3", bufs=2))
            woAp = ph.enter_context(tc.tile_pool(name="woAp", bufs=1))

            # wo rows for heads 0..7 (the 3b filler's moving operand)
            woA = woAp.tile([128, H // 2, D], BF)
            for hh in range(H // 2):
                nc.sync.dma_start(out=woA[:, hh, :],
                                  in_=wob[hh * E:(hh + 1) * E, :])

            wq2 = {}

            def emit_wq2_dma(h2):
                if h2 >= H:
                    return
                w = stage3.tile([128, KT, E], BF, tag="wq2",
                                name=f"wq2_{h2}")
                nc.sync.dma_start(
                    out=w[:],
                    in_=wqb[:, h2 * E:(h2 + 1) * E]
                    .rearrange("(kt p) e -> p kt e", p=128))
                wq2[h2] = w

            emit_wq2_dma(8)
            fill_state = {"pq2": None, "pys": None, "kt": 0, "gs": 0}
            # filler matmuls per kb slot, weighted toward the small-qc
            # (PE-light) late key blocks so exp latency stays hidden
            FILL_W = [1, 1, 1, 2, 2, 2, 3, 4]
            assert sum(FILL_W) == 16

            def emit_filler(h, kb):
                n = FILL_W[kb]
                if h < H // 2:
                    h2 = 8 + h
                    if kb == 0:
                        fill_state["pq2"] = ps_f.tile([E, QR], F32,
                                                      tag="fill",
                                                      name=f"pq2_{h2}")
                        fill_state["kt"] = 0
                    pq2 = fill_state["pq2"]
                    for _ in range(n):
                        kt = fill_state["kt"]
                        fill_state["kt"] += 1
                        nc.tensor.matmul(
                            pq2[:], wq2[h2][:, kt, :], xtq_r[:, kt, :],
                            start=(kt == 0), stop=(kt == KT - 1))
                    if kb == NKB - 1:
                        nc.vector.tensor_scalar(
                            qT[h2][:], pq2[:], bq_t[:, h2:h2 + 1], None,
                            op0=ALU.add)
                else:
                    for _ in range(n):
                        gs = fill_state["gs"]
                        fill_state["gs"] += 1
                        u, step = divmod(gs, 8)
                        oc, rt = divmod(u, RT)
                        if step == 0:
                            fill_state["pys"] = ps_f.tile([128, OC], F32,
                                                          tag="fill",
                                                          name=f"pysA_{u}")
                        pys = fill_state["pys"]
                        nc.tensor.matmul(
                            pys[:], ctxT[step][:, rt * 128:(rt + 1) * 128],
                            woA[:, step, oc * OC:(oc + 1) * OC],
                            start=(step == 0), stop=(step == 7))
                        if step == 7:
                            nc.vector.tensor_copy(
                                pA[rt][:, oc * OC:(oc + 1) * OC], pys[:])

            for h in range(H):
                kv = h % KV
                if h < H // 2:
                    emit_wq2_dma(9 + h)
                pctx = ps_c.tile([E, QR], F32, tag="pctx")
                pl = ps_l.tile([2, QR], F32, tag="pl")
                eSs = [None] * NKB
                q0s = [None] * NKB

                def emit_pl_ctx(kb):
                    eS, q0 = eSs[kb], q0s[kb]
                    qc = QR - q0
                    first = (kb == 0)
                    last = (kb == NKB - 1)
                    for st in range(ST):
                        k0 = (kb * ST + st)
                        nc.tensor.matmul(
                            pl[:, q0:], ones_kb[:, :], eS[:, st, :qc],
                            start=first and st == 0,
                            stop=last and st == ST - 1,
                            skip_group_check=True)
                        nc.tensor.matmul(
                            pctx[:, q0:], vN[:, k0, kv * E:(kv + 1) * E],
                            eS[:, st, :qc],
                            start=first and st == 0,
                            stop=last and st == ST - 1,
                            skip_group_check=True)

                for kb in range(NKB):
                    q0 = g * kb
                    qc = QR - q0
                    q0s[kb] = q0
                    pS = ps_s.tile([KSS, ST, QR], F32, tag="pS")
                    for st in range(ST):
                        k0 = kb * cfg.KB + st * KSS
                        nc.tensor.matmul(pS[:, st, :qc],
                                         kT[kv][:, k0:k0 + KSS],
                                         qT[h][:, q0:], start=True, stop=True)
                    emit_filler(h, kb)
                    # pipeline: previous block's pl/pctx go behind these
                    # scores so the PE isn't blocked on this block's exp.
                    if kb > 0:
                        emit_pl_ctx(kb - 1)
                    # additive causal mask (0/-1e9) on the diagonal window
                    nc.vector.tensor_add(pS[:, :, :g], pS[:, :, :g], mk_t[:])
                    eS = exp_p.tile([KSS, ST, QR], BF, tag="eS")
                    nc.scalar.activation(eS[:, :, :qc], pS[:, :, :qc],
                                         AF.Exp, scale=inv_sqrt_e)
                    eSs[kb] = eS
                emit_pl_ctx(NKB - 1)

                l2f = lso.tile([1, QR], F32, tag="l2f")
                nc.vector.reciprocal_approx_fast(l2f[:], pl[:1, :])
                rb = lso.tile([128, QR], F32, tag="rb")
                nc.gpsimd.partition_broadcast(rb[:], l2f[:])
                nc.vector.tensor_mul(ctxT[h][:], pctx[:], rb[:])

        xtq_stack.close()
        kvq_stack.close()

        # ===== Phase 4+5: out-proj (heads 8..15) + GELU + residual + LN ====
        # rt-outer with wo fully resident: each row tile's LayerNorm +
        # output DMA drains while the next row tile's matmuls run.
        with ExitStack() as ph:
            wop = ph.enter_context(tc.tile_pool(name="wop", bufs=1))
            rfp = ph.enter_context(tc.tile_pool(name="rfp", bufs=2))
            ps_y = ph.enter_context(tc.tile_pool(name="psy", bufs=4,
                                                 space="PSUM"))
            ep = ph.enter_context(tc.tile_pool(name="epp", bufs=3))
            stat = ph.enter_context(tc.tile_pool(name="stat", bufs=1))
            gbp = ph.enter_context(tc.tile_pool(name="gbp", bufs=1))

            woB = wop.tile([128, H // 2, D], BF)
            for hh in range(H // 2):
                h2 = H // 2 + hh
                nc.sync.dma_start(out=woB[:, hh, :],
                                  in_=wob[h2 * E:(h2 + 1) * E, :])
            if not cfg.trivial_affine:
                bo_f = gbp.tile([128, D], F32)
                gm_f = gbp.tile([128, D], F32)
                bt_f = gbp.tile([128, D], F32)
                nc.scalar.dma_start(out=bo_f[:], in_=bob[:])
                nc.scalar.dma_start(out=gm_f[:], in_=gmb[:])
                nc.scalar.dma_start(out=bt_f[:], in_=btb[:])

            for rt in range(RT):
                r_full = rfp.tile([128, D], F32, tag="rf")
                bna = stat.tile([128, NOC, 6], F32, tag="bna")
                for oc in range(NOC):
                    pys = ps_y.tile([128, OC], F32, tag="pys")
                    for hh in range(H // 2):
                        nc.tensor.matmul(
                            pys[:],
                            ctxT[H // 2 + hh][:, rt * 128:(rt + 1) * 128],
                            woB[:, hh, oc * OC:(oc + 1) * OC],
                            start=(hh == 0), stop=(hh == H // 2 - 1))
                    tb = ep.tile([128, OC], F32, tag="tb")
                    nc.vector.tensor_add(tb[:], pys[:],
                                         pA[rt][:, oc * OC:(oc + 1) * OC])
                    if not cfg.trivial_affine:
                        tb2 = ep.tile([128, OC], F32, tag="tb2")
                        nc.vector.tensor_add(
                            tb2[:], tb[:], bo_f[:, oc * OC:(oc + 1) * OC])
                        tb = tb2
                    t2 = ep.tile([128, OC], F32, tag="t2")
                    nc.scalar.activation(t2[:], tb[:], act_fn)
                    xqt = ep.tile([128, OC], F32, tag="xqt")
                    nc.scalar.dma_start(
                        out=xqt[:],
                        in_=xq[rt * 128:(rt + 1) * 128, oc * OC:(oc + 1) * OC])
                    rch = r_full[:, oc * OC:(oc + 1) * OC]
                    nc.vector.tensor_add(rch, t2[:], xqt[:])
                    nc.vector.bn_stats(bna[:, oc, :], rch)
                # stats complete for this row tile: LN + drain now, while
                # the next row tile's matmuls occupy the PE.
                mv = stat.tile([128, 2], F32, tag="mv")
                nc.vector.bn_aggr(mv[:], bna[:])
                v_e = stat.tile([128, 1], F32, tag="ve")
                nc.vector.tensor_scalar_add(v_e[:], mv[:, 1:2], 1e-5)
                # Newton rsqrt on vector only (no ACT table switch):
                # y0 = 1.09545 - 0.1895*v, then 4x y *= 1.5 - 0.5*v*y^2
                y = stat.tile([128, 1], F32, tag="y")
                nc.vector.tensor_scalar(y[:], v_e[:], -0.1895,
                                        1.09545, op0=ALU.mult, op1=ALU.add)
                for _ in range(4):
                    h2t = stat.tile([128, 1], F32, tag="h2t")
                    nc.vector.tensor_mul(h2t[:], y[:], y[:])
                    nc.vector.tensor_mul(h2t[:], h2t[:], v_e[:])
                    nc.vector.tensor_scalar(h2t[:], h2t[:], -0.5, 1.5,
                                            op0=ALU.mult, op1=ALU.add)
                    nc.vector.tensor_mul(y[:], y[:], h2t[:])
                nmr = stat.tile([128, 1], F32, tag="nmr")
                nc.vector.scalar_tensor_tensor(
                    nmr[:], mv[:, 0:1], -1.0, y[:],
                    op0=ALU.mult, op1=ALU.mult)
                yfull = ep.tile([128, D], F32, tag="yfull", bufs=2)
                for c in range(NOC):
                    slc = slice(c * OC, (c + 1) * OC)
                    if cfg.trivial_affine:
                        nc.vector.tensor_scalar(
                            yfull[:, slc], r_full[:, slc], y[:], nmr[:],
                            op0=ALU.mult, op1=ALU.add)
                    else:
                        yf = ep.tile([128, OC], F32, tag="yf")
                        nc.vector.tensor_scalar(
                            yf[:], r_full[:, slc], y[:], nmr[:],
                            op0=ALU.mult, op1=ALU.add)
                        y2 = ep.tile([128, OC], F32, tag="y2")
                        nc.vector.tensor_mul(y2[:], yf[:], gm_f[:, slc])
                        nc.vector.tensor_add(yfull[:, slc], y2[:],
                                             bt_f[:, slc])
                # single full-width DMA: 8KB-contiguous rows, far fewer
                # packets than per-chunk writes
                eng = (nc.gpsimd, nc.scalar, nc.sync, nc.gpsimd)[rt]
                eng.dma_start(out=out[rt * 128:(rt + 1) * 128, :],
                              in_=yfull[:])

    nc.finalize()
    return nc


# ---------------------------------------------------------------------------
# host-side staging + sharding
# ---------------------------------------------------------------------------

def _bf16(a):
    import ml_dtypes
    return np.ascontiguousarray(np.asarray(a, np.float32)).astype(
        ml_dtypes.bfloat16)


def build_mask01(cfg: Cfg, j: int):
    # mk01[c, r] = 0 iff key (c = st*128 + k) is visible to the r-th query
    # of the diagonal block (c <= 64*j + r), else -1e9; same for every kb.
    c = np.arange(cfg.KB)[:, None]
    r = np.arange(cfg.g)[None, :]
    return np.where(c <= j * cfg.g + r, 0.0, -1.0e9).astype(np.float32)


def q_rows(cfg: Cfg, j: int):
    g = cfg.g
    return np.concatenate(
        [np.arange((j + 4 * i) * g, (j + 4 * i + 1) * g) for i in range(8)])


def make_in_map(cfg: Cfg, shared, xb_T_f32, xb_f32, j):
    rows = q_rows(cfg, j)
    return dict(
        shared,
        xtb=xb_T_f32,
        xtqb=np.ascontiguousarray(_bf16(xb_T_f32[:, rows])),
        xq=np.ascontiguousarray(xb_f32[rows]),
        mk01=build_mask01(cfg, j),
    )


def make_shared(cfg: Cfg, Wq, bq, Wk, bk, Wv, bv, Wo, bo, gamma, beta):
    H, KV, E, D = cfg.H, cfg.KV, cfg.E, cfg.D
    shared = {
        "wqb": _bf16(Wq),
        "wkb": np.ascontiguousarray(Wk, dtype=np.float32),
        "wvb": np.ascontiguousarray(Wv, dtype=np.float32),
        "wob": _bf16(Wo),
        "bqT": np.ascontiguousarray(
            np.asarray(bq, np.float32).reshape(H, E).T),
        "bkT": np.ascontiguousarray(
            np.asarray(bk, np.float32).reshape(KV, E).T),
        "bvb": np.ascontiguousarray(
            np.broadcast_to(np.asarray(bv, np.float32), (128, KV * E))),
    }
    if not cfg.trivial_affine:
        shared["bob"] = np.ascontiguousarray(
            np.broadcast_to(np.asarray(bo, np.float32), (128, D)))
        shared["gmb"] = np.ascontiguousarray(
            np.broadcast_to(np.asarray(gamma, np.float32), (128, D)))
        shared["btb"] = np.ascontiguousarray(
            np.broadcast_to(np.asarray(beta, np.float32), (128, D)))
    return shared


def assemble(cfg: Cfg, results, B):
    out = np.empty((B, cfg.L, cfg.D), np.float32)
    for core in range(4 * B):
        b, j = divmod(core, 4)
        out[b, q_rows(cfg, j)] = results[core]["out"]
    return out


_NC_CACHE = {}


def kernel(x, Wq, bq, Wk, bk, Wv, bv, Wo, bo, gamma, beta):
    from concourse.bass_utils import run_bass_kernel_spmd

    trivial = bool(
        np.all(np.asarray(gamma) == 1.0) and np.all(np.asarray(beta) == 0.0)
        and np.all(np.asarray(bo) == 0.0))
    cfg = Cfg(trivial_affine=trivial)
    if cfg not in _NC_CACHE:
        _NC_CACHE[cfg] = build_program(cfg)
    nc = _NC_CACHE[cfg]
    shared = make_shared(cfg, Wq, bq, Wk, bk, Wv, bv, Wo, bo, gamma, beta)
    x = np.asarray(x, np.float32)
    xT = [np.ascontiguousarray(x[b].T) for b in range(2)]
    in_maps = [make_in_map(cfg, shared, xT[core // 4], x[core // 4],
                           core % 4)
               for core in range(8)]
    res = run_bass_kernel_spmd(nc, in_maps, list(range(8)))
    return assemble(cfg, res.results, 2)


# revision 16
# speedup vs baseline: 1.1147x; 1.0498x over previous
"""Trainium2 Bass kernel for nn_Attention_Layer_78855599554595.

GQA attention layer: QKV proj -> causal GQA attention (16 heads, 4 kv heads,
E=128) -> out proj -> exact GELU -> residual -> LayerNorm.  B=2, L=2048, D=2048.

Sharding: zero-communication interleaved sequence parallelism.
  - 8 cores = 2 batches x 4 cores/batch.
  - Core j of a batch owns query rows in g=64-row blocks strided by 4:
    global blocks {j, j+4, ..., j+28} (512 rows).  For key block kb
    (256 keys), query-buffer columns [64*kb, 512) attend to it; the
    64-wide window at the start is the diagonal (host-supplied 0/1
    multiplicative mask, identical for every kb).
  - Each core computes K/V for its full batch (redundant 4x; on-chip
    collectives are slower than the 55us of PE time they would save).

v3 design (v2's trace showed the PE starving during the scalar-bound
attention phase, HAM-throttling to 1.2GHz; fixed by interleaving):
  - Q-projection of heads 8-15 is interleaved into heads 0-7's
    attention; out-proj partial sums over heads 0-7 are interleaved
    into heads 8-15's attention.  The PE never idles, stays at 2.4GHz,
    and half the out-projection rides under the exp shadow.
  - The causal mask is an additive -1e9 add on the vector engine on
    the PSUM scores (pre-exp), off the slow gpsimd path.

v2 design (from trace analysis of the 542us v1):
  - All matmuls run bf16 x bf16 (fp32 PSUM accumulate).  Host stages
    x^T and all weights as bf16: halves DMA, kills every fp32->fp32r
    tensor_copy, and removes the fp32r small-free-dim 4x penalty so the
    causal q-window never needs clamping.
  - K^T is produced directly by the projection matmul (Wk tile
    stationary, x^T moving) - no PE transposes, no scalar evictions.
  - K/V projection streams kt (contraction) innermost in 4 row-groups:
    first matmul needs only 1 weight tile + 1 x tile (~2us), not 8MB.
  - Softmax: exp in one scalar.activation per (head, key block) over
    both key subtiles ([128,2,qc] PSUM read); causal mask applied
    after exp as a 0/1 multiply on the gpsimd engine (SBUF-only);
    1/l broadcast via gpsimd.partition_broadcast (no PE broadcast
    matmul, no scalar copy).
  - Out-proj + GELU + residual + LN stats (vector bn_stats/bn_aggr)
    are fused per (oc, rt) tile; rstd via a vector-only Newton rsqrt
    (no scalar Sqrt -> no ACT table-set ping-pong); the LayerNorm +
    output DMA for each row tile drains right after its last column
    chunk, eliminating the 40us serial tail.

Host-side (free): transposes/gathers, bf16 staging, mask construction.
"""

import sys

sys.path.insert(0, "/opt/trn_rl_repo")

import numpy as np

from contextlib import ExitStack
from dataclasses import dataclass

from concourse import bacc, mybir, tile

F32 = mybir.dt.float32
R = mybir.dt.float32r
BF = mybir.dt.bfloat16
AF = mybir.ActivationFunctionType
ALU = mybir.AluOpType


@dataclass(frozen=True)
class Cfg:
    L: int = 2048          # sequence length (per batch)
    D: int = 2048          # model dim
    H: int = 16            # query heads
    KV: int = 4            # kv heads
    E: int = 128           # head dim (= partition width)
    trivial_affine: bool = False  # gamma==1, beta==0, bo==0: skip those ops
    act: object = None     # None -> exact GELU (CoreSim lacks Gelu; tests
                           # may override with an implemented fn like Tanh)

    @property
    def g(self):           # q block granularity (32 blocks across L)
        return self.L // 32

    @property
    def KB(self):          # key block size
        return self.L // 8

    @property
    def KSS(self):         # key subtile (partition) size
        return min(self.KB, 128)

    @property
    def ST(self):          # key subtiles per key block
        return max(1, self.KB // 128)

    @property
    def QR(self):          # query rows per core
        return self.L // 4

    @property
    def KT(self):          # contraction tiles over D
        return self.D // 128

    @property
    def RT(self):          # 128-row tiles of the core's q rows
        return self.QR // 128

    @property
    def OC(self):          # out-proj / LN column chunk
        return min(self.D, 512)


def build_program(cfg: Cfg):
    """Build the single-core SPMD Bass program. Returns finalized nc."""
    L, D, H, KV, E = cfg.L, cfg.D, cfg.H, cfg.KV, cfg.E
    g, KSS, ST, QR, KT, RT, OC = (cfg.g, cfg.KSS, cfg.ST, cfg.QR, cfg.KT,
                                  cfg.RT, cfg.OC)
    NOC = D // OC
    NKB = L // cfg.KB      # 8 key blocks
    inv_sqrt_e = 1.0 / float(np.sqrt(E))
    act_fn = cfg.act if cfg.act is not None else AF.Gelu

    nc = bacc.Bacc(None, target_bir_lowering=False)

    # ---- DRAM I/O (per-core data; same names on every core) ----
    xtb = nc.dram_tensor("xtb", [D, L], R, kind="ExternalInput")      # x[b].T
    xtqb = nc.dram_tensor("xtqb", [D, QR], BF, kind="ExternalInput")  # q cols
    xq = nc.dram_tensor("xq", [QR, D], F32, kind="ExternalInput")     # residual
    wqb = nc.dram_tensor("wqb", [D, H * E], BF, kind="ExternalInput")
    wkb = nc.dram_tensor("wkb", [D, KV * E], R, kind="ExternalInput")
    wvb = nc.dram_tensor("wvb", [D, KV * E], R, kind="ExternalInput")
    wob = nc.dram_tensor("wob", [H * E, D], BF, kind="ExternalInput")
    bqT = nc.dram_tensor("bqT", [E, H], F32, kind="ExternalInput")
    bkT = nc.dram_tensor("bkT", [E, KV], F32, kind="ExternalInput")
    bvb = nc.dram_tensor("bvb", [128, KV * E], F32, kind="ExternalInput")
    # additive diagonal mask (0 / -1e9) in S^T layout: [key (2x128), q (64)]
    mk01 = nc.dram_tensor("mk01", [cfg.KB, g], F32, kind="ExternalInput")
    if not cfg.trivial_affine:
        bob = nc.dram_tensor("bob", [128, D], F32, kind="ExternalInput")
        gmb = nc.dram_tensor("gmb", [128, D], F32, kind="ExternalInput")
        btb = nc.dram_tensor("btb", [128, D], F32, kind="ExternalInput")
    out = nc.dram_tensor("out", [QR, D], F32, kind="ExternalOutput")

    with tile.TileContext(nc) as tc, ExitStack() as top:
        const = top.enter_context(tc.tile_pool(name="const", bufs=1))
        ctxp = top.enter_context(tc.tile_pool(name="ctxp", bufs=1))
        pap = top.enter_context(tc.tile_pool(name="pap", bufs=1))
        kvq_stack = ExitStack()
        kvq = kvq_stack.enter_context(tc.tile_pool(name="kvq", bufs=1))
        qtp = kvq_stack.enter_context(tc.tile_pool(name="qtp", bufs=1))

        # ---- constants (vector DMA queue; issued at t=0) ----
        ones_kb = const.tile([128, 2], BF)
        nc.gpsimd.memset(ones_kb[:], 1.0)
        bq_t = const.tile([E, H], F32)
        bk_t = const.tile([E, KV], F32)
        bv_t = const.tile([128, KV * E], F32)
        mk_t = const.tile([KSS, ST, g], F32)
        nc.gpsimd.dma_start(out=bq_t[:], in_=bqT[:])
        nc.gpsimd.dma_start(out=bk_t[:], in_=bkT[:])
        nc.gpsimd.dma_start(out=bv_t[:], in_=bvb[:])
        nc.gpsimd.dma_start(out=mk_t[:],
                            in_=mk01.rearrange("(s p) q -> p s q", p=KSS))

        # persistent activations
        kT = [kvq.tile([E, L], BF, tag=f"kT{kv}", name=f"kT{kv}")
              for kv in range(KV)]
        vN = kvq.tile([KSS, L // KSS, KV * E], BF)      # V natural, key-major
        qT = [qtp.tile([E, QR], BF, tag=f"qT{h}", name=f"qT{h}")
              for h in range(H)]
        ctxT = [ctxp.tile([E, QR], BF, tag=f"cT{h}", name=f"cT{h}")
                for h in range(H)]
        # out-proj partial sums over heads 0..7 (filled during phase 3b)
        pA = [pap.tile([128, D], F32, tag=f"pA{rt}", name=f"pA{rt}")
              for rt in range(RT)]

        # phase-2 moving operand: resident x^T at q rows (vector queue, t=0)
        xtq_stack = ExitStack()
        xtqp = xtq_stack.enter_context(tc.tile_pool(name="xtqp", bufs=1))
        xtq_r = xtqp.tile([128, KT, QR], BF)

        def emit_xtq_loads():   # called after phase-1 group 0
            for kt in range(KT):
                nc.scalar.dma_start(out=xtq_r[:, kt, :],
                                    in_=xtqb[kt * 128:(kt + 1) * 128, :])

        # ---- PE warm-up: ~6us of dependency-free matmuls so HAM is at
        # full clock by the time the first weights/x tiles arrive ----
        with ExitStack() as ph:
            wrm_p = ph.enter_context(tc.tile_pool(name="wrmp", bufs=1))
            ps_w = ph.enter_context(tc.tile_pool(name="psw", bufs=1,
                                                 space="PSUM"))
            wrm = wrm_p.tile([128, 512], BF)
            nc.gpsimd.memset(wrm[:], 0.0)
            pwrm = ps_w.tile([2, 512], F32)
            for i in range(16):
                nc.tensor.matmul(pwrm[:], ones_kb[:, :], wrm[:],
                                 start=(i == 0), stop=(i == 15))

        # ================= Phase 1: K/V projections ========================
        # kT direct:  out[E, rows] += Wk_tile.T @ xT_tile   (moving = x^T)
        # V natural:  out[rows, kvE] += xT_tile.T @ Wv_tile (moving = Wv)
        with ExitStack() as ph:
            wkv = ph.enter_context(tc.tile_pool(name="wkv", bufs=1))
            stage = ph.enter_context(tc.tile_pool(name="stage1", bufs=4))
            ps1 = ph.enter_context(tc.tile_pool(name="ps1", bufs=1,
                                                space="PSUM"))
            wk_r = wkv.tile([128, KT, KV * E], R)
            wv_r = wkv.tile([128, KT, KV * E], R)
            for kt in range(KT):
                nc.sync.dma_start(out=wk_r[:, kt, :],
                                  in_=wkb[kt * 128:(kt + 1) * 128, :])
                nc.sync.dma_start(out=wv_r[:, kt, :],
                                  in_=wvb[kt * 128:(kt + 1) * 128, :])

            NG = L // 512
            for g4 in range(NG):
                c0 = 512 * g4
                pKT = [ps1.tile([E, 512], F32, tag=f"pKT{kv}",
                                name=f"pKT{kv}") for kv in range(KV)]
                pV = [ps1.tile([128, KV * E], F32, tag=f"pV{c}",
                               name=f"pV{c}") for c in range(4)]
                for kt in range(KT):
                    xg = stage.tile([128, 512], R, tag="xg")
                    eng = nc.gpsimd if kt % 2 == 0 else nc.scalar
                    eng.dma_start(
                        out=xg[:], in_=xtb[kt * 128:(kt + 1) * 128,
                                           c0:c0 + 512])
                    for kv in range(KV):
                        nc.tensor.matmul(
                            pKT[kv][:], wk_r[:, kt, kv * E:(kv + 1) * E],
                            xg[:], start=(kt == 0), stop=(kt == KT - 1))
                    for c in range(4):
                        nc.tensor.matmul(
                            pV[c][:], xg[:, c * 128:(c + 1) * 128],
                            wv_r[:, kt, :], start=(kt == 0),
                            stop=(kt == KT - 1))
                for kv in range(KV):
                    nc.vector.tensor_scalar(
                        kT[kv][:, c0:c0 + 512], pKT[kv][:],
                        bk_t[:, kv:kv + 1], None, op0=ALU.add)
                for c in range(4):
                    nc.vector.tensor_add(vN[:, g4 * 4 + c, :], pV[c][:],
                                         bv_t[:])
                if g4 == 0:
                    emit_xtq_loads()

        # ====== Phase 2a: Q^T projection, heads 0..7 =======================
        # (heads 8..15 are projected inside phase 3a as PE filler work)
        with ExitStack() as ph:
            stage = ph.enter_context(tc.tile_pool(name="stage2", bufs=8))
            ps2 = ph.enter_context(tc.tile_pool(name="ps2", bufs=1,
                                                space="PSUM"))
            HB = 8
            pqs = [ps2.tile([E, QR], F32, tag=f"pq{hh}", name=f"pq{hh}")
                   for hh in range(HB)]
            for kt in range(KT):
                wqs = stage.tile([128, HB, E], BF, tag="wqs")
                nc.sync.dma_start(
                    out=wqs[:],
                    in_=wqb[kt * 128:(kt + 1) * 128, 0:HB * E]
                    .rearrange("p (h e) -> p h e", h=HB))
                for hh in range(HB):
                    nc.tensor.matmul(
                        pqs[hh][:], wqs[:, hh, :], xtq_r[:, kt, :],
                        start=(kt == 0), stop=(kt == KT - 1))
            for hh in range(HB):
                nc.vector.tensor_scalar(
                    qT[hh][:], pqs[hh][:], bq_t[:, hh:hh + 1], None,
                    op0=ALU.add)

        # ================= Phase 3: attention ==============================
        # The exp stream on the scalar engine is the phase bottleneck, so
        # the PE is kept busy (and HAM-warm) with interleaved filler:
        #   heads 0..7:  Q-projection of head 8+h (2 matmuls / kb slot)
        #   heads 8..15: out-proj partial sums over heads 0..7 into pA
        with ExitStack() as ph:
            ps_s = ph.enter_context(tc.tile_pool(name="pss", bufs=2,
                                                 space="PSUM"))
            ps_c = ph.enter_context(tc.tile_pool(name="psc", bufs=2,
                                                 space="PSUM"))
            ps_l = ph.enter_context(tc.tile_pool(name="psl", bufs=1,
                                                 space="PSUM"))
            ps_f = ph.enter_context(tc.tile_pool(name="psf", bufs=1,
                                                 space="PSUM"))
            exp_p = ph.enter_context(tc.tile_pool(name="expp", bufs=3))
            lso = ph.enter_context(tc.tile_pool(name="lso", bufs=2))
            stage3 = ph.enter_context(tc.tile_pool(name="stage3", bufs=2))
            woAp = ph.enter_context(tc.tile_pool(name="woAp", bufs=1))

            # wo rows for heads 0..7 (the 3b filler's moving operand)
            woA = woAp.tile([128, H // 2, D], BF)
            for hh in range(H // 2):
                nc.sync.dma_start(out=woA[:, hh, :],
                                  in_=wob[hh * E:(hh + 1) * E, :])

            wq2 = {}

            def emit_wq2_dma(h2):
                if h2 >= H:
                    return
                w = stage3.tile([128, KT, E], BF, tag="wq2",
                                name=f"wq2_{h2}")
                nc.sync.dma_start(
                    out=w[:],
                    in_=wqb[:, h2 * E:(h2 + 1) * E]
                    .rearrange("(kt p) e -> p kt e", p=128))
                wq2[h2] = w

            emit_wq2_dma(8)
            fill_state = {"pq2": None, "pys": None, "kt": 0, "gs": 0}
            # filler matmuls per kb slot, weighted toward the small-qc
            # (PE-light) late key blocks so exp latency stays hidden
            FILL_W = [1, 1, 1, 2, 2, 2, 3, 4]
            assert sum(FILL_W) == 16

            def emit_filler(h, kb):
                n = FILL_W[kb]
                if h < H // 2:
                    h2 = 8 + h
                    if kb == 0:
                        fill_state["pq2"] = ps_f.tile([E, QR], F32,
                                                      tag="fill",
                                                      name=f"pq2_{h2}")
                        fill_state["kt"] = 0
                    pq2 = fill_state["pq2"]
                    for _ in range(n):
                        kt = fill_state["kt"]
                        fill_state["kt"] += 1
                        nc.tensor.matmul(
                            pq2[:], wq2[h2][:, kt, :], xtq_r[:, kt, :],
                            start=(kt == 0), stop=(kt == KT - 1))
                    if kb == NKB - 1:
                        nc.vector.tensor_scalar(
                            qT[h2][:], pq2[:], bq_t[:, h2:h2 + 1], None,
                            op0=ALU.add)
                else:
                    for _ in range(n):
                        gs = fill_state["gs"]
                        fill_state["gs"] += 1
                        u, step = divmod(gs, 8)
                        oc, rt = divmod(u, RT)
                        if step == 0:
                            fill_state["pys"] = ps_f.tile([128, OC], F32,
                                                          tag="fill",
                                                          name=f"pysA_{u}")
                        pys = fill_state["pys"]
                        nc.tensor.matmul(
                            pys[:], ctxT[step][:, rt * 128:(rt + 1) * 128],
                            woA[:, step, oc * OC:(oc + 1) * OC],
                            start=(step == 0), stop=(step == 7))
                        if step == 7:
                            nc.vector.tensor_copy(
                                pA[rt][:, oc * OC:(oc + 1) * OC], pys[:])

            for h in range(H):
                kv = h % KV
                if h < H // 2:
                    emit_wq2_dma(9 + h)
                pctx = ps_c.tile([E, QR], F32, tag="pctx")
                pl = ps_l.tile([2, QR], F32, tag="pl")
                eSs = [None] * NKB
                q0s = [None] * NKB

                def emit_pl_ctx(kb):
                    eS, q0 = eSs[kb], q0s[kb]
                    qc = QR - q0
                    first = (kb == 0)
                    last = (kb == NKB - 1)
                    for st in range(ST):
                        k0 = (kb * ST + st)
                        nc.tensor.matmul(
                            pl[:, q0:], ones_kb[:, :], eS[:, st, :qc],
                            start=first and st == 0,
                            stop=last and st == ST - 1,
                            skip_group_check=True)
                        nc.tensor.matmul(
                            pctx[:, q0:], vN[:, k0, kv * E:(kv + 1) * E],
                            eS[:, st, :qc],
                            start=first and st == 0,
                            stop=last and st == ST - 1,
                            skip_group_check=True)

                for kb in range(NKB):
                    q0 = g * kb
                    qc = QR - q0
                    q0s[kb] = q0
                    pS = ps_s.tile([KSS, ST, QR], F32, tag="pS")
                    for st in range(ST):
                        k0 = kb * cfg.KB + st * KSS
                        nc.tensor.matmul(pS[:, st, :qc],
                                         kT[kv][:, k0:k0 + KSS],
                                         qT[h][:, q0:], start=True, stop=True)
                    emit_filler(h, kb)
                    # pipeline: previous block's pl/pctx go behind these
                    # scores so the PE isn't blocked on this block's exp.
                    if kb > 0:
                        emit_pl_ctx(kb - 1)
                    # additive causal mask (0/-1e9) on the diagonal window
                    nc.vector.tensor_add(pS[:, :, :g], pS[:, :, :g], mk_t[:])
                    eS = exp_p.tile([KSS, ST, QR], BF, tag="eS")
                    nc.scalar.activation(eS[:, :, :qc], pS[:, :, :qc],
                                         AF.Exp, scale=inv_sqrt_e)
                    eSs[kb] = eS
                emit_pl_ctx(NKB - 1)

                l2f = lso.tile([1, QR], F32, tag="l2f")
                nc.vector.reciprocal_approx_fast(l2f[:], pl[:1, :])
                rb = lso.tile([128, QR], F32, tag="rb")
                nc.gpsimd.partition_broadcast(rb[:], l2f[:])
                nc.vector.tensor_mul(ctxT[h][:], pctx[:], rb[:])

        xtq_stack.close()
        kvq_stack.close()

        # ===== Phase 4+5: out-proj (heads 8..15) + GELU + residual + LN ====
        # rt-outer with wo fully resident: each row tile's LayerNorm +
        # output DMA drains while the next row tile's matmuls run.
        with ExitStack() as ph:
            wop = ph.enter_context(tc.tile_pool(name="wop", bufs=1))
            rfp = ph.enter_context(tc.tile_pool(name="rfp", bufs=2))
            ps_y = ph.enter_context(tc.tile_pool(name="psy", bufs=4,
                                                 space="PSUM"))
            ep = ph.enter_context(tc.tile_pool(name="epp", bufs=3))
            stat = ph.enter_context(tc.tile_pool(name="stat", bufs=1))
            gbp = ph.enter_context(tc.tile_pool(name="gbp", bufs=1))

            woB = wop.tile([128, H // 2, D], BF)
            for hh in range(H // 2):
                h2 = H // 2 + hh
                nc.sync.dma_start(out=woB[:, hh, :],
                                  in_=wob[h2 * E:(h2 + 1) * E, :])
            xqr = [ep.tile([128, D], F32, tag=f"xqr{rt}", name=f"xqr{rt}",
                           bufs=1) for rt in range(RT)]
            for rt in range(RT):
                nc.scalar.dma_start(out=xqr[rt][:],
                                    in_=xq[rt * 128:(rt + 1) * 128, :])
            if not cfg.trivial_affine:
                bo_f = gbp.tile([128, D], F32)
                gm_f = gbp.tile([128, D], F32)
                bt_f = gbp.tile([128, D], F32)
                nc.scalar.dma_start(out=bo_f[:], in_=bob[:])
                nc.scalar.dma_start(out=gm_f[:], in_=gmb[:])
                nc.scalar.dma_start(out=bt_f[:], in_=btb[:])

            for rt in range(RT):
                r_full = rfp.tile([128, D], F32, tag="rf")
                bna = stat.tile([128, NOC, 6], F32, tag="bna")
                for oc in range(NOC):
                    pys = ps_y.tile([128, OC], F32, tag="pys")
                    for hh in range(H // 2):
                        nc.tensor.matmul(
                            pys[:],
                            ctxT[H // 2 + hh][:, rt * 128:(rt + 1) * 128],
                            woB[:, hh, oc * OC:(oc + 1) * OC],
                            start=(hh == 0), stop=(hh == H // 2 - 1))
                    tb = ep.tile([128, OC], F32, tag="tb")
                    nc.vector.tensor_add(tb[:], pys[:],
                                         pA[rt][:, oc * OC:(oc + 1) * OC])
                    if not cfg.trivial_affine:
                        tb2 = ep.tile([128, OC], F32, tag="tb2")
                        nc.vector.tensor_add(
                            tb2[:], tb[:], bo_f[:, oc * OC:(oc + 1) * OC])
                        tb = tb2
                    t2 = ep.tile([128, OC], F32, tag="t2")
                    nc.scalar.activation(t2[:], tb[:], act_fn)
                    rch = r_full[:, oc * OC:(oc + 1) * OC]
                    nc.vector.tensor_add(rch, t2[:],
                                         xqr[rt][:, oc * OC:(oc + 1) * OC])
                    nc.vector.bn_stats(bna[:, oc, :], rch)
                # stats complete for this row tile: LN + drain now, while
                # the next row tile's matmuls occupy the PE.
                mv = stat.tile([128, 2], F32, tag="mv")
                nc.vector.bn_aggr(mv[:], bna[:])
                v_e = stat.tile([128, 1], F32, tag="ve")
                nc.vector.tensor_scalar_add(v_e[:], mv[:, 1:2], 1e-5)
                # Newton rsqrt on vector only (no ACT table switch):
                # y0 = 1.09545 - 0.1895*v, then 4x y *= 1.5 - 0.5*v*y^2
                y = stat.tile([128, 1], F32, tag="y")
                nc.vector.tensor_scalar(y[:], v_e[:], -0.1895,
                                        1.09545, op0=ALU.mult, op1=ALU.add)
                for _ in range(4):
                    h2t = stat.tile([128, 1], F32, tag="h2t")
                    nc.vector.tensor_mul(h2t[:], y[:], y[:])
                    nc.vector.tensor_mul(h2t[:], h2t[:], v_e[:])
                    nc.vector.tensor_scalar(h2t[:], h2t[:], -0.5, 1.5,
                                            op0=ALU.mult, op1=ALU.add)
                    nc.vector.tensor_mul(y[:], y[:], h2t[:])
                nmr = stat.tile([128, 1], F32, tag="nmr")
                nc.vector.scalar_tensor_tensor(
                    nmr[:], mv[:, 0:1], -1.0, y[:],
                    op0=ALU.mult, op1=ALU.mult)
                yfull = ep.tile([128, D], F32, tag="yfull", bufs=2)
                for c in range(NOC):
                    slc = slice(c * OC, (c + 1) * OC)
                    if cfg.trivial_affine:
                        nc.vector.tensor_scalar(
                            yfull[:, slc], r_full[:, slc], y[:], nmr[:],
                            op0=ALU.mult, op1=ALU.add)
                    else:
                        yf = ep.tile([128, OC], F32, tag="yf")
                        nc.vector.tensor_scalar(
                            yf[:], r_full[:, slc], y[:], nmr[:],
                            op0=ALU.mult, op1=ALU.add)
                        y2 = ep.tile([128, OC], F32, tag="y2")
                        nc.vector.tensor_mul(y2[:], yf[:], gm_f[:, slc])
                        nc.vector.tensor_add(yfull[:, slc], y2[:],
                                             bt_f[:, slc])
                # two half-width DMAs on different queues: 4KB-contiguous
                # rows, twice the drain parallelism
                e0, e1 = ((nc.gpsimd, nc.sync), (nc.scalar, nc.gpsimd),
                          (nc.sync, nc.scalar), (nc.gpsimd, nc.sync))[rt]
                hD = D // 2
                e0.dma_start(out=out[rt * 128:(rt + 1) * 128, :hD],
                             in_=yfull[:, :hD])
                e1.dma_start(out=out[rt * 128:(rt + 1) * 128, hD:],
                             in_=yfull[:, hD:])

    nc.finalize()
    return nc


# ---------------------------------------------------------------------------
# host-side staging + sharding
# ---------------------------------------------------------------------------

def _bf16(a):
    import ml_dtypes
    return np.ascontiguousarray(np.asarray(a, np.float32)).astype(
        ml_dtypes.bfloat16)


def build_mask01(cfg: Cfg, j: int):
    # mk01[c, r] = 0 iff key (c = st*128 + k) is visible to the r-th query
    # of the diagonal block (c <= 64*j + r), else -1e9; same for every kb.
    c = np.arange(cfg.KB)[:, None]
    r = np.arange(cfg.g)[None, :]
    return np.where(c <= j * cfg.g + r, 0.0, -1.0e9).astype(np.float32)


def q_rows(cfg: Cfg, j: int):
    g = cfg.g
    return np.concatenate(
        [np.arange((j + 4 * i) * g, (j + 4 * i + 1) * g) for i in range(8)])


def make_in_map(cfg: Cfg, shared, xb_T_f32, xb_f32, j):
    rows = q_rows(cfg, j)
    return dict(
        shared,
        xtb=xb_T_f32,
        xtqb=np.ascontiguousarray(_bf16(xb_T_f32[:, rows])),
        xq=np.ascontiguousarray(xb_f32[rows]),
        mk01=build_mask01(cfg, j),
    )


def make_shared(cfg: Cfg, Wq, bq, Wk, bk, Wv, bv, Wo, bo, gamma, beta):
    H, KV, E, D = cfg.H, cfg.KV, cfg.E, cfg.D
    shared = {
        "wqb": _bf16(Wq),
        "wkb": np.ascontiguousarray(Wk, dtype=np.float32),
        "wvb": np.ascontiguousarray(Wv, dtype=np.float32),
        "wob": _bf16(Wo),
        "bqT": np.ascontiguousarray(
            np.asarray(bq, np.float32).reshape(H, E).T),
        "bkT": np.ascontiguousarray(
            np.asarray(bk, np.float32).reshape(KV, E).T),
        "bvb": np.ascontiguousarray(
            np.broadcast_to(np.asarray(bv, np.float32), (128, KV * E))),
    }
    if not cfg.trivial_affine:
        shared["bob"] = np.ascontiguousarray(
            np.broadcast_to(np.asarray(bo, np.float32), (128, D)))
        shared["gmb"] = np.ascontiguousarray(
            np.broadcast_to(np.asarray(gamma, np.float32), (128, D)))
        shared["btb"] = np.ascontiguousarray(
            np.broadcast_to(np.asarray(beta, np.float32), (128, D)))
    return shared


def assemble(cfg: Cfg, results, B):
    out = np.empty((B, cfg.L, cfg.D), np.float32)
    for core in range(4 * B):
        b, j = divmod(core, 4)
        out[b, q_rows(cfg, j)] = results[core]["out"]
    return out


_NC_CACHE = {}


def kernel(x, Wq, bq, Wk, bk, Wv, bv, Wo, bo, gamma, beta):
    from concourse.bass_utils import run_bass_kernel_spmd

    trivial = bool(
        np.all(np.asarray(gamma) == 1.0) and np.all(np.asarray(beta) == 0.0)
        and np.all(np.asarray(bo) == 0.0))
    cfg = Cfg(trivial_affine=trivial)
    if cfg not in _NC_CACHE:
        _NC_CACHE[cfg] = build_program(cfg)
    nc = _NC_CACHE[cfg]
    shared = make_shared(cfg, Wq, bq, Wk, bk, Wv, bv, Wo, bo, gamma, beta)
    x = np.asarray(x, np.float32)
    xT = [np.ascontiguousarray(x[b].T) for b in range(2)]
    in_maps = [make_in_map(cfg, shared, xT[core // 4], x[core // 4],
                           core % 4)
               for core in range(8)]
    res = run_bass_kernel_spmd(nc, in_maps, list(range(8)))
    return assemble(cfg, res.results, 2)


# revision 17
# speedup vs baseline: 1.1192x; 1.0040x over previous
"""Trainium2 Bass kernel for nn_Attention_Layer_78855599554595.

GQA attention layer: QKV proj -> causal GQA attention (16 heads, 4 kv heads,
E=128) -> out proj -> exact GELU -> residual -> LayerNorm.  B=2, L=2048, D=2048.

Sharding: zero-communication interleaved sequence parallelism.
  - 8 cores = 2 batches x 4 cores/batch.
  - Core j of a batch owns query rows in g=64-row blocks strided by 4:
    global blocks {j, j+4, ..., j+28} (512 rows).  For key block kb
    (256 keys), query-buffer columns [64*kb, 512) attend to it; the
    64-wide window at the start is the diagonal (host-supplied 0/1
    multiplicative mask, identical for every kb).
  - Each core computes K/V for its full batch (redundant 4x; on-chip
    collectives are slower than the 55us of PE time they would save).

v3 design (v2's trace showed the PE starving during the scalar-bound
attention phase, HAM-throttling to 1.2GHz; fixed by interleaving):
  - Q-projection of heads 8-15 is interleaved into heads 0-7's
    attention; out-proj partial sums over heads 0-7 are interleaved
    into heads 8-15's attention.  The PE never idles, stays at 2.4GHz,
    and half the out-projection rides under the exp shadow.
  - The causal mask is an additive -1e9 add on the vector engine on
    the PSUM scores (pre-exp), off the slow gpsimd path.

v2 design (from trace analysis of the 542us v1):
  - All matmuls run bf16 x bf16 (fp32 PSUM accumulate).  Host stages
    x^T and all weights as bf16: halves DMA, kills every fp32->fp32r
    tensor_copy, and removes the fp32r small-free-dim 4x penalty so the
    causal q-window never needs clamping.
  - K^T is produced directly by the projection matmul (Wk tile
    stationary, x^T moving) - no PE transposes, no scalar evictions.
  - K/V projection streams kt (contraction) innermost in 4 row-groups:
    first matmul needs only 1 weight tile + 1 x tile (~2us), not 8MB.
  - Softmax: exp in one scalar.activation per (head, key block) over
    both key subtiles ([128,2,qc] PSUM read); causal mask applied
    after exp as a 0/1 multiply on the gpsimd engine (SBUF-only);
    1/l broadcast via gpsimd.partition_broadcast (no PE broadcast
    matmul, no scalar copy).
  - Out-proj + GELU + residual + LN stats (vector bn_stats/bn_aggr)
    are fused per (oc, rt) tile; rstd via a vector-only Newton rsqrt
    (no scalar Sqrt -> no ACT table-set ping-pong); the LayerNorm +
    output DMA for each row tile drains right after its last column
    chunk, eliminating the 40us serial tail.

Host-side (free): transposes/gathers, bf16 staging, mask construction.
"""

import sys

sys.path.insert(0, "/opt/trn_rl_repo")

import numpy as np

from contextlib import ExitStack
from dataclasses import dataclass

from concourse import bacc, mybir, tile

F32 = mybir.dt.float32
R = mybir.dt.float32r
BF = mybir.dt.bfloat16
AF = mybir.ActivationFunctionType
ALU = mybir.AluOpType


@dataclass(frozen=True)
class Cfg:
    L: int = 2048          # sequence length (per batch)
    D: int = 2048          # model dim
    H: int = 16            # query heads
    KV: int = 4            # kv heads
    E: int = 128           # head dim (= partition width)
    trivial_affine: bool = False  # gamma==1, beta==0, bo==0: skip those ops
    act: object = None     # None -> exact GELU (CoreSim lacks Gelu; tests
                           # may override with an implemented fn like Tanh)

    @property
    def g(self):           # q block granularity (32 blocks across L)
        return self.L // 32

    @property
    def KB(self):          # key block size
        return self.L // 8

    @property
    def KSS(self):         # key subtile (partition) size
        return min(self.KB, 128)

    @property
    def ST(self):          # key subtiles per key block
        return max(1, self.KB // 128)

    @property
    def QR(self):          # query rows per core
        return self.L // 4

    @property
    def KT(self):          # contraction tiles over D
        return self.D // 128

    @property
    def RT(self):          # 128-row tiles of the core's q rows
        return self.QR // 128

    @property
    def OC(self):          # out-proj / LN column chunk
        return min(self.D, 512)


def build_program(cfg: Cfg):
    """Build the single-core SPMD Bass program. Returns finalized nc."""
    L, D, H, KV, E = cfg.L, cfg.D, cfg.H, cfg.KV, cfg.E
    g, KSS, ST, QR, KT, RT, OC = (cfg.g, cfg.KSS, cfg.ST, cfg.QR, cfg.KT,
                                  cfg.RT, cfg.OC)
    NOC = D // OC
    NKB = L // cfg.KB      # 8 key blocks
    inv_sqrt_e = 1.0 / float(np.sqrt(E))
    act_fn = cfg.act if cfg.act is not None else AF.Gelu

    nc = bacc.Bacc(None, target_bir_lowering=False)

    # ---- DRAM I/O (per-core data; same names on every core) ----
    xtb = nc.dram_tensor("xtb", [D, L], R, kind="ExternalInput")      # x[b].T
    xtqb = nc.dram_tensor("xtqb", [D, QR], BF, kind="ExternalInput")  # q cols
    xq = nc.dram_tensor("xq", [QR, D], F32, kind="ExternalInput")     # residual
    wqb = nc.dram_tensor("wqb", [D, H * E], BF, kind="ExternalInput")
    wkb = nc.dram_tensor("wkb", [D, KV * E], R, kind="ExternalInput")
    wvb = nc.dram_tensor("wvb", [D, KV * E], R, kind="ExternalInput")
    wob = nc.dram_tensor("wob", [H * E, D], BF, kind="ExternalInput")
    bqT = nc.dram_tensor("bqT", [E, H], F32, kind="ExternalInput")
    bkT = nc.dram_tensor("bkT", [E, KV], F32, kind="ExternalInput")
    bvb = nc.dram_tensor("bvb", [128, KV * E], F32, kind="ExternalInput")
    # additive diagonal mask (0 / -1e9) in S^T layout: [key (2x128), q (64)]
    mk01 = nc.dram_tensor("mk01", [cfg.KB, g], F32, kind="ExternalInput")
    if not cfg.trivial_affine:
        bob = nc.dram_tensor("bob", [128, D], F32, kind="ExternalInput")
        gmb = nc.dram_tensor("gmb", [128, D], F32, kind="ExternalInput")
        btb = nc.dram_tensor("btb", [128, D], F32, kind="ExternalInput")
    out = nc.dram_tensor("out", [QR, D], F32, kind="ExternalOutput")

    with tile.TileContext(nc) as tc, ExitStack() as top:
        const = top.enter_context(tc.tile_pool(name="const", bufs=1))
        ctxp = top.enter_context(tc.tile_pool(name="ctxp", bufs=1))
        pap = top.enter_context(tc.tile_pool(name="pap", bufs=1))
        kvq_stack = ExitStack()
        kvq = kvq_stack.enter_context(tc.tile_pool(name="kvq", bufs=1))
        qtp = kvq_stack.enter_context(tc.tile_pool(name="qtp", bufs=1))

        # ---- constants (vector DMA queue; issued at t=0) ----
        ones_kb = const.tile([128, 2], BF)
        nc.gpsimd.memset(ones_kb[:], 1.0)
        bq_t = const.tile([E, H], F32)
        bk_t = const.tile([E, KV], F32)
        bv_t = const.tile([128, KV * E], F32)
        mk_t = const.tile([KSS, ST, g], F32)
        nc.gpsimd.dma_start(out=bq_t[:], in_=bqT[:])
        nc.gpsimd.dma_start(out=bk_t[:], in_=bkT[:])
        nc.gpsimd.dma_start(out=bv_t[:], in_=bvb[:])
        nc.gpsimd.dma_start(out=mk_t[:],
                            in_=mk01.rearrange("(s p) q -> p s q", p=KSS))

        # persistent activations
        kT = [kvq.tile([E, L], BF, tag=f"kT{kv}", name=f"kT{kv}")
              for kv in range(KV)]
        vN = kvq.tile([KSS, L // KSS, KV * E], BF)      # V natural, key-major
        qT = [qtp.tile([E, QR], BF, tag=f"qT{h}", name=f"qT{h}")
              for h in range(H)]
        ctxT = [ctxp.tile([E, QR], BF, tag=f"cT{h}", name=f"cT{h}")
                for h in range(H)]
        # out-proj partial sums over heads 0..7 (filled during phase 3b)
        pA = [pap.tile([128, D], F32, tag=f"pA{rt}", name=f"pA{rt}")
              for rt in range(RT)]

        # phase-2 moving operand: resident x^T at q rows (vector queue, t=0)
        xtq_stack = ExitStack()
        xtqp = xtq_stack.enter_context(tc.tile_pool(name="xtqp", bufs=1))
        xtq_r = xtqp.tile([128, KT, QR], BF)

        def emit_xtq_loads():   # called after phase-1 group 0
            for kt in range(KT):
                nc.scalar.dma_start(out=xtq_r[:, kt, :],
                                    in_=xtqb[kt * 128:(kt + 1) * 128, :])

        # ---- PE warm-up: ~6us of dependency-free matmuls so HAM is at
        # full clock by the time the first weights/x tiles arrive ----
        with ExitStack() as ph:
            wrm_p = ph.enter_context(tc.tile_pool(name="wrmp", bufs=1))
            ps_w = ph.enter_context(tc.tile_pool(name="psw", bufs=1,
                                                 space="PSUM"))
            wrm = wrm_p.tile([128, 512], BF)
            nc.gpsimd.memset(wrm[:], 0.0)
            pwrm = ps_w.tile([2, 512], F32)
            for i in range(16):
                nc.tensor.matmul(pwrm[:], ones_kb[:, :], wrm[:],
                                 start=(i == 0), stop=(i == 15))

        # ================= Phase 1: K/V projections ========================
        # kT direct:  out[E, rows] += Wk_tile.T @ xT_tile   (moving = x^T)
        # V natural:  out[rows, kvE] += xT_tile.T @ Wv_tile (moving = Wv)
        with ExitStack() as ph:
            wkv = ph.enter_context(tc.tile_pool(name="wkv", bufs=1))
            stage = ph.enter_context(tc.tile_pool(name="stage1", bufs=4))
            ps1 = ph.enter_context(tc.tile_pool(name="ps1", bufs=1,
                                                space="PSUM"))
            wk_r = wkv.tile([128, KT, KV * E], R)
            wv_r = wkv.tile([128, KT, KV * E], R)
            for kt in range(KT):
                nc.sync.dma_start(out=wk_r[:, kt, :],
                                  in_=wkb[kt * 128:(kt + 1) * 128, :])
                nc.sync.dma_start(out=wv_r[:, kt, :],
                                  in_=wvb[kt * 128:(kt + 1) * 128, :])

            NG = L // 512
            for g4 in range(NG):
                c0 = 512 * g4
                pKT = [ps1.tile([E, 512], F32, tag=f"pKT{kv}",
                                name=f"pKT{kv}") for kv in range(KV)]
                pV = [ps1.tile([128, KV * E], F32, tag=f"pV{c}",
                               name=f"pV{c}") for c in range(4)]
                for kt in range(KT):
                    xg = stage.tile([128, 512], R, tag="xg")
                    eng = nc.gpsimd if kt % 2 == 0 else nc.scalar
                    eng.dma_start(
                        out=xg[:], in_=xtb[kt * 128:(kt + 1) * 128,
                                           c0:c0 + 512])
                    for kv in range(KV):
                        nc.tensor.matmul(
                            pKT[kv][:], wk_r[:, kt, kv * E:(kv + 1) * E],
                            xg[:], start=(kt == 0), stop=(kt == KT - 1))
                    for c in range(4):
                        nc.tensor.matmul(
                            pV[c][:], xg[:, c * 128:(c + 1) * 128],
                            wv_r[:, kt, :], start=(kt == 0),
                            stop=(kt == KT - 1))
                for kv in range(KV):
                    nc.vector.tensor_scalar(
                        kT[kv][:, c0:c0 + 512], pKT[kv][:],
                        bk_t[:, kv:kv + 1], None, op0=ALU.add)
                for c in range(4):
                    nc.vector.tensor_add(vN[:, g4 * 4 + c, :], pV[c][:],
                                         bv_t[:])
                if g4 == 0:
                    emit_xtq_loads()

        # ====== Phase 2a: Q^T projection, heads 0..7 =======================
        # (heads 8..15 are projected inside phase 3a as PE filler work)
        with ExitStack() as ph:
            stage = ph.enter_context(tc.tile_pool(name="stage2", bufs=8))
            ps2 = ph.enter_context(tc.tile_pool(name="ps2", bufs=1,
                                                space="PSUM"))
            HB = 8
            pqs = [ps2.tile([E, QR], F32, tag=f"pq{hh}", name=f"pq{hh}")
                   for hh in range(HB)]
            for kt in range(KT):
                wqs = stage.tile([128, HB, E], BF, tag="wqs")
                nc.sync.dma_start(
                    out=wqs[:],
                    in_=wqb[kt * 128:(kt + 1) * 128, 0:HB * E]
                    .rearrange("p (h e) -> p h e", h=HB))
                for hh in range(HB):
                    nc.tensor.matmul(
                        pqs[hh][:], wqs[:, hh, :], xtq_r[:, kt, :],
                        start=(kt == 0), stop=(kt == KT - 1))
            for hh in range(HB):
                nc.vector.tensor_scalar(
                    qT[hh][:], pqs[hh][:], bq_t[:, hh:hh + 1], None,
                    op0=ALU.add)

        # ================= Phase 3: attention ==============================
        # The exp stream on the scalar engine is the phase bottleneck, so
        # the PE is kept busy (and HAM-warm) with interleaved filler:
        #   heads 0..7:  Q-projection of head 8+h (2 matmuls / kb slot)
        #   heads 8..15: out-proj partial sums over heads 0..7 into pA
        with ExitStack() as ph:
            ps_s = ph.enter_context(tc.tile_pool(name="pss", bufs=2,
                                                 space="PSUM"))
            ps_c = ph.enter_context(tc.tile_pool(name="psc", bufs=2,
                                                 space="PSUM"))
            ps_l = ph.enter_context(tc.tile_pool(name="psl", bufs=1,
                                                 space="PSUM"))
            ps_f = ph.enter_context(tc.tile_pool(name="psf", bufs=1,
                                                 space="PSUM"))
            exp_p = ph.enter_context(tc.tile_pool(name="expp", bufs=3))
            lso = ph.enter_context(tc.tile_pool(name="lso", bufs=2))
            stage3 = ph.enter_context(tc.tile_pool(name="stage3", bufs=2))
            woAp = ph.enter_context(tc.tile_pool(name="woAp", bufs=1))

            # wo rows for heads 0..7 (the 3b filler's moving operand)
            woA = woAp.tile([128, H // 2, D], BF)
            for hh in range(H // 2):
                nc.sync.dma_start(out=woA[:, hh, :],
                                  in_=wob[hh * E:(hh + 1) * E, :])

            wq2 = {}

            def emit_wq2_dma(h2):
                if h2 >= H:
                    return
                w = stage3.tile([128, KT, E], BF, tag="wq2",
                                name=f"wq2_{h2}")
                nc.sync.dma_start(
                    out=w[:],
                    in_=wqb[:, h2 * E:(h2 + 1) * E]
                    .rearrange("(kt p) e -> p kt e", p=128))
                wq2[h2] = w

            emit_wq2_dma(8)
            fill_state = {"pq2": None, "pys": None, "kt": 0, "gs": 0}
            # filler matmuls per kb slot, weighted toward the small-qc
            # (PE-light) late key blocks so exp latency stays hidden
            FILL_W = [1, 1, 1, 2, 2, 2, 3, 4]
            assert sum(FILL_W) == 16

            def emit_filler(h, kb):
                n = FILL_W[kb]
                if h < H // 2:
                    h2 = 8 + h
                    if kb == 0:
                        fill_state["pq2"] = ps_f.tile([E, QR], F32,
                                                      tag="fill",
                                                      name=f"pq2_{h2}")
                        fill_state["kt"] = 0
                    pq2 = fill_state["pq2"]
                    for _ in range(n):
                        kt = fill_state["kt"]
                        fill_state["kt"] += 1
                        nc.tensor.matmul(
                            pq2[:], wq2[h2][:, kt, :], xtq_r[:, kt, :],
                            start=(kt == 0), stop=(kt == KT - 1))
                    if kb == NKB - 1:
                        nc.vector.tensor_scalar(
                            qT[h2][:], pq2[:], bq_t[:, h2:h2 + 1], None,
                            op0=ALU.add)
                else:
                    for _ in range(n):
                        gs = fill_state["gs"]
                        fill_state["gs"] += 1
                        u, step = divmod(gs, 8)
                        oc, rt = divmod(u, RT)
                        if step == 0:
                            fill_state["pys"] = ps_f.tile([128, OC], F32,
                                                          tag="fill",
                                                          name=f"pysA_{u}")
                        pys = fill_state["pys"]
                        nc.tensor.matmul(
                            pys[:], ctxT[step][:, rt * 128:(rt + 1) * 128],
                            woA[:, step, oc * OC:(oc + 1) * OC],
                            start=(step == 0), stop=(step == 7))
                        if step == 7:
                            nc.vector.tensor_copy(
                                pA[rt][:, oc * OC:(oc + 1) * OC], pys[:])

            for h in range(H):
                kv = h % KV
                if h < H // 2:
                    emit_wq2_dma(9 + h)
                pctx = ps_c.tile([E, QR], F32, tag="pctx")
                pl = ps_l.tile([2, QR], F32, tag="pl")
                eSs = [None] * NKB
                q0s = [None] * NKB

                def emit_pl_ctx(kb):
                    eS, q0 = eSs[kb], q0s[kb]
                    qc = QR - q0
                    first = (kb == 0)
                    last = (kb == NKB - 1)
                    for st in range(ST):
                        k0 = (kb * ST + st)
                        nc.tensor.matmul(
                            pl[:, q0:], ones_kb[:, :], eS[:, st, :qc],
                            start=first and st == 0,
                            stop=last and st == ST - 1,
                            skip_group_check=True)
                        nc.tensor.matmul(
                            pctx[:, q0:], vN[:, k0, kv * E:(kv + 1) * E],
                            eS[:, st, :qc],
                            start=first and st == 0,
                            stop=last and st == ST - 1,
                            skip_group_check=True)

                for kb in range(NKB):
                    q0 = g * kb
                    qc = QR - q0
                    q0s[kb] = q0
                    pS = ps_s.tile([KSS, ST, QR], F32, tag="pS")
                    for st in range(ST):
                        k0 = kb * cfg.KB + st * KSS
                        nc.tensor.matmul(pS[:, st, :qc],
                                         kT[kv][:, k0:k0 + KSS],
                                         qT[h][:, q0:], start=True, stop=True)
                    emit_filler(h, kb)
                    # pipeline: previous block's pl/pctx go behind these
                    # scores so the PE isn't blocked on this block's exp.
                    if kb > 0:
                        emit_pl_ctx(kb - 1)
                    # additive causal mask (0/-1e9) on the diagonal window
                    nc.vector.tensor_add(pS[:, :, :g], pS[:, :, :g], mk_t[:])
                    eS = exp_p.tile([KSS, ST, QR], BF, tag="eS")
                    nc.scalar.activation(eS[:, :, :qc], pS[:, :, :qc],
                                         AF.Exp, scale=inv_sqrt_e)
                    eSs[kb] = eS
                emit_pl_ctx(NKB - 1)

                l2f = lso.tile([1, QR], F32, tag="l2f")
                nc.vector.reciprocal_approx_fast(l2f[:], pl[:1, :])
                rb = lso.tile([128, QR], F32, tag="rb")
                nc.gpsimd.partition_broadcast(rb[:], l2f[:])
                nc.vector.tensor_mul(ctxT[h][:], pctx[:], rb[:])

        xtq_stack.close()
        kvq_stack.close()

        # ===== Phase 4+5: out-proj (heads 8..15) + GELU + residual + LN ====
        # rt-outer with wo fully resident: each row tile's LayerNorm +
        # output DMA drains while the next row tile's matmuls run.
        with ExitStack() as ph:
            wop = ph.enter_context(tc.tile_pool(name="wop", bufs=1))
            rfp = ph.enter_context(tc.tile_pool(name="rfp", bufs=2))
            ps_y = ph.enter_context(tc.tile_pool(name="psy", bufs=4,
                                                 space="PSUM"))
            ep = ph.enter_context(tc.tile_pool(name="epp", bufs=3))
            stat = ph.enter_context(tc.tile_pool(name="stat", bufs=1))
            gbp = ph.enter_context(tc.tile_pool(name="gbp", bufs=1))

            woB = wop.tile([128, H // 2, D], BF)
            # oc-major chunk order so (rt0, oc0) only waits for the first MB
            for oc0 in range(NOC):
                for pc in range(2):
                    h0 = H // 2 + pc * 4
                    nc.sync.dma_start(
                        out=woB[:, pc * 4:(pc + 1) * 4,
                                oc0 * OC:(oc0 + 1) * OC],
                        in_=wob[h0 * E:(h0 + 4) * E, oc0 * OC:(oc0 + 1) * OC]
                        .rearrange("(h p) c -> p h c", p=128))
            xqr = [ep.tile([128, D], F32, tag=f"xqr{rt}", name=f"xqr{rt}",
                           bufs=1) for rt in range(RT)]
            for rt in range(RT):
                nc.scalar.dma_start(out=xqr[rt][:],
                                    in_=xq[rt * 128:(rt + 1) * 128, :])
            if not cfg.trivial_affine:
                bo_f = gbp.tile([128, D], F32)
                gm_f = gbp.tile([128, D], F32)
                bt_f = gbp.tile([128, D], F32)
                nc.scalar.dma_start(out=bo_f[:], in_=bob[:])
                nc.scalar.dma_start(out=gm_f[:], in_=gmb[:])
                nc.scalar.dma_start(out=bt_f[:], in_=btb[:])

            for rt in range(RT):
                r_full = rfp.tile([128, D], F32, tag="rf")
                bna = stat.tile([128, NOC, 6], F32, tag="bna")
                for oc in range(NOC):
                    pys = ps_y.tile([128, OC], F32, tag="pys")
                    for hh in range(H // 2):
                        nc.tensor.matmul(
                            pys[:],
                            ctxT[H // 2 + hh][:, rt * 128:(rt + 1) * 128],
                            woB[:, hh, oc * OC:(oc + 1) * OC],
                            start=(hh == 0), stop=(hh == H // 2 - 1))
                    tb = ep.tile([128, OC], F32, tag="tb")
                    nc.vector.tensor_add(tb[:], pys[:],
                                         pA[rt][:, oc * OC:(oc + 1) * OC])
                    if not cfg.trivial_affine:
                        tb2 = ep.tile([128, OC], F32, tag="tb2")
                        nc.vector.tensor_add(
                            tb2[:], tb[:], bo_f[:, oc * OC:(oc + 1) * OC])
                        tb = tb2
                    t2 = ep.tile([128, OC], F32, tag="t2")
                    nc.scalar.activation(t2[:], tb[:], act_fn)
                    rch = r_full[:, oc * OC:(oc + 1) * OC]
                    nc.vector.tensor_add(rch, t2[:],
                                         xqr[rt][:, oc * OC:(oc + 1) * OC])
                    nc.vector.bn_stats(bna[:, oc, :], rch)
                # stats complete for this row tile: LN + drain now, while
                # the next row tile's matmuls occupy the PE.
                mv = stat.tile([128, 2], F32, tag="mv")
                nc.vector.bn_aggr(mv[:], bna[:])
                v_e = stat.tile([128, 1], F32, tag="ve")
                nc.vector.tensor_scalar_add(v_e[:], mv[:, 1:2], 1e-5)
                # Newton rsqrt on vector only (no ACT table switch):
                # y0 = 1.09545 - 0.1895*v, then 4x y *= 1.5 - 0.5*v*y^2
                y = stat.tile([128, 1], F32, tag="y")
                nc.vector.tensor_scalar(y[:], v_e[:], -0.1895,
                                        1.09545, op0=ALU.mult, op1=ALU.add)
                for _ in range(4):
                    h2t = stat.tile([128, 1], F32, tag="h2t")
                    nc.vector.tensor_mul(h2t[:], y[:], y[:])
                    nc.vector.tensor_mul(h2t[:], h2t[:], v_e[:])
                    nc.vector.tensor_scalar(h2t[:], h2t[:], -0.5, 1.5,
                                            op0=ALU.mult, op1=ALU.add)
                    nc.vector.tensor_mul(y[:], y[:], h2t[:])
                nmr = stat.tile([128, 1], F32, tag="nmr")
                nc.vector.scalar_tensor_tensor(
                    nmr[:], mv[:, 0:1], -1.0, y[:],
                    op0=ALU.mult, op1=ALU.mult)
                yfull = ep.tile([128, D], F32, tag="yfull", bufs=2)
                for c in range(NOC):
                    slc = slice(c * OC, (c + 1) * OC)
                    if cfg.trivial_affine:
                        nc.vector.tensor_scalar(
                            yfull[:, slc], r_full[:, slc], y[:], nmr[:],
                            op0=ALU.mult, op1=ALU.add)
                    else:
                        yf = ep.tile([128, OC], F32, tag="yf")
                        nc.vector.tensor_scalar(
                            yf[:], r_full[:, slc], y[:], nmr[:],
                            op0=ALU.mult, op1=ALU.add)
                        y2 = ep.tile([128, OC], F32, tag="y2")
                        nc.vector.tensor_mul(y2[:], yf[:], gm_f[:, slc])
                        nc.vector.tensor_add(yfull[:, slc], y2[:],
                                             bt_f[:, slc])
                # two half-width DMAs on different queues: 4KB-contiguous
                # rows, twice the drain parallelism
                e0, e1 = ((nc.gpsimd, nc.sync), (nc.scalar, nc.gpsimd),
                          (nc.sync, nc.scalar), (nc.gpsimd, nc.sync))[rt]
                hD = D // 2
                e0.dma_start(out=out[rt * 128:(rt + 1) * 128, :hD],
                             in_=yfull[:, :hD])
                e1.dma_start(out=out[rt * 128:(rt + 1) * 128, hD:],
                             in_=yfull[:, hD:])

    nc.finalize()
    return nc


# ---------------------------------------------------------------------------
# host-side staging + sharding
# ---------------------------------------------------------------------------

def _bf16(a):
    import ml_dtypes
    return np.ascontiguousarray(np.asarray(a, np.float32)).astype(
        ml_dtypes.bfloat16)


def build_mask01(cfg: Cfg, j: int):
    # mk01[c, r] = 0 iff key (c = st*128 + k) is visible to the r-th query
    # of the diagonal block (c <= 64*j + r), else -1e9; same for every kb.
    c = np.arange(cfg.KB)[:, None]
    r = np.arange(cfg.g)[None, :]
    return np.where(c <= j * cfg.g + r, 0.0, -1.0e9).astype(np.float32)


def q_rows(cfg: Cfg, j: int):
    g = cfg.g
    return np.concatenate(
        [np.arange((j + 4 * i) * g, (j + 4 * i + 1) * g) for i in range(8)])


def make_in_map(cfg: Cfg, shared, xb_T_f32, xb_f32, j):
    rows = q_rows(cfg, j)
    return dict(
        shared,
        xtb=xb_T_f32,
        xtqb=np.ascontiguousarray(_bf16(xb_T_f32[:, rows])),
        xq=np.ascontiguousarray(xb_f32[rows]),
        mk01=build_mask01(cfg, j),
    )


def make_shared(cfg: Cfg, Wq, bq, Wk, bk, Wv, bv, Wo, bo, gamma, beta):
    H, KV, E, D = cfg.H, cfg.KV, cfg.E, cfg.D
    shared = {
        "wqb": _bf16(Wq),
        "wkb": np.ascontiguousarray(Wk, dtype=np.float32),
        "wvb": np.ascontiguousarray(Wv, dtype=np.float32),
        "wob": _bf16(Wo),
        "bqT": np.ascontiguousarray(
            np.asarray(bq, np.float32).reshape(H, E).T),
        "bkT": np.ascontiguousarray(
            np.asarray(bk, np.float32).reshape(KV, E).T),
        "bvb": np.ascontiguousarray(
            np.broadcast_to(np.asarray(bv, np.float32), (128, KV * E))),
    }
    if not cfg.trivial_affine:
        shared["bob"] = np.ascontiguousarray(
            np.broadcast_to(np.asarray(bo, np.float32), (128, D)))
        shared["gmb"] = np.ascontiguousarray(
            np.broadcast_to(np.asarray(gamma, np.float32), (128, D)))
        shared["btb"] = np.ascontiguousarray(
            np.broadcast_to(np.asarray(beta, np.float32), (128, D)))
    return shared


def assemble(cfg: Cfg, results, B):
    out = np.empty((B, cfg.L, cfg.D), np.float32)
    for core in range(4 * B):
        b, j = divmod(core, 4)
        out[b, q_rows(cfg, j)] = results[core]["out"]
    return out


_NC_CACHE = {}


def kernel(x, Wq, bq, Wk, bk, Wv, bv, Wo, bo, gamma, beta):
    from concourse.bass_utils import run_bass_kernel_spmd

    trivial = bool(
        np.all(np.asarray(gamma) == 1.0) and np.all(np.asarray(beta) == 0.0)
        and np.all(np.asarray(bo) == 0.0))
    cfg = Cfg(trivial_affine=trivial)
    if cfg not in _NC_CACHE:
        _NC_CACHE[cfg] = build_program(cfg)
    nc = _NC_CACHE[cfg]
    shared = make_shared(cfg, Wq, bq, Wk, bk, Wv, bv, Wo, bo, gamma, beta)
    x = np.asarray(x, np.float32)
    xT = [np.ascontiguousarray(x[b].T) for b in range(2)]
    in_maps = [make_in_map(cfg, shared, xT[core // 4], x[core // 4],
                           core % 4)
               for core in range(8)]
    res = run_bass_kernel_spmd(nc, in_maps, list(range(8)))
    return assemble(cfg, res.results, 2)


# revision 18
# speedup vs baseline: 1.1363x; 1.0153x over previous
"""Trainium2 Bass kernel for nn_Attention_Layer_78855599554595.

GQA attention layer: QKV proj -> causal GQA attention (16 heads, 4 kv heads,
E=128) -> out proj -> exact GELU -> residual -> LayerNorm.  B=2, L=2048, D=2048.

Sharding: zero-communication interleaved sequence parallelism.
  - 8 cores = 2 batches x 4 cores/batch.
  - Core j of a batch owns query rows in g=64-row blocks strided by 4:
    global blocks {j, j+4, ..., j+28} (512 rows).  For key block kb
    (256 keys), query-buffer columns [64*kb, 512) attend to it; the
    64-wide window at the start is the diagonal (host-supplied 0/1
    multiplicative mask, identical for every kb).
  - Each core computes K/V for its full batch (redundant 4x; on-chip
    collectives are slower than the 55us of PE time they would save).

v3 design (v2's trace showed the PE starving during the scalar-bound
attention phase, HAM-throttling to 1.2GHz; fixed by interleaving):
  - Q-projection of heads 8-15 is interleaved into heads 0-7's
    attention; out-proj partial sums over heads 0-7 are interleaved
    into heads 8-15's attention.  The PE never idles, stays at 2.4GHz,
    and half the out-projection rides under the exp shadow.
  - The causal mask is an additive -1e9 add on the vector engine on
    the PSUM scores (pre-exp), off the slow gpsimd path.

v2 design (from trace analysis of the 542us v1):
  - All matmuls run bf16 x bf16 (fp32 PSUM accumulate).  Host stages
    x^T and all weights as bf16: halves DMA, kills every fp32->fp32r
    tensor_copy, and removes the fp32r small-free-dim 4x penalty so the
    causal q-window never needs clamping.
  - K^T is produced directly by the projection matmul (Wk tile
    stationary, x^T moving) - no PE transposes, no scalar evictions.
  - K/V projection streams kt (contraction) innermost in 4 row-groups:
    first matmul needs only 1 weight tile + 1 x tile (~2us), not 8MB.
  - Softmax: exp in one scalar.activation per (head, key block) over
    both key subtiles ([128,2,qc] PSUM read); causal mask applied
    after exp as a 0/1 multiply on the gpsimd engine (SBUF-only);
    1/l broadcast via gpsimd.partition_broadcast (no PE broadcast
    matmul, no scalar copy).
  - Out-proj + GELU + residual + LN stats (vector bn_stats/bn_aggr)
    are fused per (oc, rt) tile; rstd via a vector-only Newton rsqrt
    (no scalar Sqrt -> no ACT table-set ping-pong); the LayerNorm +
    output DMA for each row tile drains right after its last column
    chunk, eliminating the 40us serial tail.

Host-side (free): transposes/gathers, bf16 staging, mask construction.
"""

import sys

sys.path.insert(0, "/opt/trn_rl_repo")

import numpy as np

from contextlib import ExitStack
from dataclasses import dataclass

from concourse import bacc, mybir, tile

F32 = mybir.dt.float32
R = mybir.dt.float32r
BF = mybir.dt.bfloat16
AF = mybir.ActivationFunctionType
ALU = mybir.AluOpType


@dataclass(frozen=True)
class Cfg:
    L: int = 2048          # sequence length (per batch)
    D: int = 2048          # model dim
    H: int = 16            # query heads
    KV: int = 4            # kv heads
    E: int = 128           # head dim (= partition width)
    trivial_affine: bool = False  # gamma==1, beta==0, bo==0: skip those ops
    act: object = None     # None -> exact GELU (CoreSim lacks Gelu; tests
                           # may override with an implemented fn like Tanh)

    @property
    def g(self):           # q block granularity (32 blocks across L)
        return self.L // 32

    @property
    def KB(self):          # key block size
        return self.L // 8

    @property
    def KSS(self):         # key subtile (partition) size
        return min(self.KB, 128)

    @property
    def ST(self):          # key subtiles per key block
        return max(1, self.KB // 128)

    @property
    def QR(self):          # query rows per core
        return self.L // 4

    @property
    def KT(self):          # contraction tiles over D
        return self.D // 128

    @property
    def RT(self):          # 128-row tiles of the core's q rows
        return self.QR // 128

    @property
    def OC(self):          # out-proj / LN column chunk
        return min(self.D, 512)


def build_program(cfg: Cfg):
    """Build the single-core SPMD Bass program. Returns finalized nc."""
    L, D, H, KV, E = cfg.L, cfg.D, cfg.H, cfg.KV, cfg.E
    g, KSS, ST, QR, KT, RT, OC = (cfg.g, cfg.KSS, cfg.ST, cfg.QR, cfg.KT,
                                  cfg.RT, cfg.OC)
    NOC = D // OC
    NKB = L // cfg.KB      # 8 key blocks
    inv_sqrt_e = 1.0 / float(np.sqrt(E))
    act_fn = cfg.act if cfg.act is not None else AF.Gelu

    nc = bacc.Bacc(None, target_bir_lowering=False)

    # ---- DRAM I/O (per-core data; same names on every core) ----
    xtb = nc.dram_tensor("xtb", [D, L], R, kind="ExternalInput")      # x[b].T
    xtqb = nc.dram_tensor("xtqb", [D, QR], BF, kind="ExternalInput")  # q cols
    xq = nc.dram_tensor("xq", [QR, D], F32, kind="ExternalInput")     # residual
    wqb = nc.dram_tensor("wqb", [D, H * E], BF, kind="ExternalInput")
    wkb = nc.dram_tensor("wkb", [D, KV * E], R, kind="ExternalInput")
    wvb = nc.dram_tensor("wvb", [D, KV * E], R, kind="ExternalInput")
    wob = nc.dram_tensor("wob", [H * E, D], BF, kind="ExternalInput")
    bqT = nc.dram_tensor("bqT", [E, H], F32, kind="ExternalInput")
    bkT = nc.dram_tensor("bkT", [E, KV], F32, kind="ExternalInput")
    bvb = nc.dram_tensor("bvb", [128, KV * E], F32, kind="ExternalInput")
    # additive diagonal mask (0 / -1e9) in S^T layout: [key (2x128), q (64)]
    mk01 = nc.dram_tensor("mk01", [cfg.KB, g], F32, kind="ExternalInput")
    if not cfg.trivial_affine:
        bob = nc.dram_tensor("bob", [128, D], F32, kind="ExternalInput")
        gmb = nc.dram_tensor("gmb", [128, D], F32, kind="ExternalInput")
        btb = nc.dram_tensor("btb", [128, D], F32, kind="ExternalInput")
    out = nc.dram_tensor("out", [QR, D], F32, kind="ExternalOutput")

    with tile.TileContext(nc) as tc, ExitStack() as top:
        const = top.enter_context(tc.tile_pool(name="const", bufs=1))
        ctxp = top.enter_context(tc.tile_pool(name="ctxp", bufs=1))
        pap = top.enter_context(tc.tile_pool(name="pap", bufs=1))
        kvq_stack = ExitStack()
        kvq = kvq_stack.enter_context(tc.tile_pool(name="kvq", bufs=1))
        qtp = kvq_stack.enter_context(tc.tile_pool(name="qtp", bufs=1))

        # ---- constants (vector DMA queue; issued at t=0) ----
        ones_kb = const.tile([128, 2], BF)
        nc.gpsimd.memset(ones_kb[:], 1.0)
        bq_t = const.tile([E, H], F32)
        bk_t = const.tile([E, KV], F32)
        bv_t = const.tile([128, KV * E], F32)
        mk_t = const.tile([KSS, ST, g], F32)
        nc.gpsimd.dma_start(out=bq_t[:], in_=bqT[:])
        nc.gpsimd.dma_start(out=bk_t[:], in_=bkT[:])
        nc.gpsimd.dma_start(out=bv_t[:], in_=bvb[:])
        nc.gpsimd.dma_start(out=mk_t[:],
                            in_=mk01.rearrange("(s p) q -> p s q", p=KSS))

        # persistent activations
        kT = [kvq.tile([E, L], BF, tag=f"kT{kv}", name=f"kT{kv}")
              for kv in range(KV)]
        vN = kvq.tile([KSS, L // KSS, KV * E], BF)      # V natural, key-major
        qT = [qtp.tile([E, QR], BF, tag=f"qT{h}", name=f"qT{h}")
              for h in range(H)]
        ctxT = [ctxp.tile([E, QR], BF, tag=f"cT{h}", name=f"cT{h}")
                for h in range(H)]
        # out-proj partial sums over heads 0..7 (filled during phase 3b)
        pA = [pap.tile([128, D], F32, tag=f"pA{rt}", name=f"pA{rt}")
              for rt in range(RT)]

        # phase-2 moving operand: resident x^T at q rows (vector queue, t=0)
        xtq_stack = ExitStack()
        xtqp = xtq_stack.enter_context(tc.tile_pool(name="xtqp", bufs=1))
        xtq_r = xtqp.tile([128, KT, QR], BF)

        def emit_xtq_loads():   # called after phase-1 group 0
            for kt in range(KT):
                nc.scalar.dma_start(out=xtq_r[:, kt, :],
                                    in_=xtqb[kt * 128:(kt + 1) * 128, :])

        # ---- PE warm-up: ~6us of dependency-free matmuls so HAM is at
        # full clock by the time the first weights/x tiles arrive ----
        with ExitStack() as ph:
            wrm_p = ph.enter_context(tc.tile_pool(name="wrmp", bufs=1))
            ps_w = ph.enter_context(tc.tile_pool(name="psw", bufs=1,
                                                 space="PSUM"))
            wrm = wrm_p.tile([128, 512], BF)
            nc.gpsimd.memset(wrm[:], 0.0)
            pwrm = ps_w.tile([2, 512], F32)
            for i in range(16):
                nc.tensor.matmul(pwrm[:], ones_kb[:, :], wrm[:],
                                 start=(i == 0), stop=(i == 15))

        # ================= Phase 1: K/V projections ========================
        # kT direct:  out[E, rows] += Wk_tile.T @ xT_tile   (moving = x^T)
        # V natural:  out[rows, kvE] += xT_tile.T @ Wv_tile (moving = Wv)
        with ExitStack() as ph:
            wkv = ph.enter_context(tc.tile_pool(name="wkv", bufs=1))
            stage = ph.enter_context(tc.tile_pool(name="stage1", bufs=8))
            ps1 = ph.enter_context(tc.tile_pool(name="ps1", bufs=1,
                                                space="PSUM"))
            wk_r = wkv.tile([128, KT, KV * E], R)
            wv_r = wkv.tile([128, KT, KV * E], R)
            for kt in range(KT):
                nc.sync.dma_start(out=wk_r[:, kt, :],
                                  in_=wkb[kt * 128:(kt + 1) * 128, :])
                nc.sync.dma_start(out=wv_r[:, kt, :],
                                  in_=wvb[kt * 128:(kt + 1) * 128, :])

            NG = L // 512
            for g4 in range(NG):
                c0 = 512 * g4
                pKT = [ps1.tile([E, 512], F32, tag=f"pKT{kv}",
                                name=f"pKT{kv}") for kv in range(KV)]
                pV = [ps1.tile([128, KV * E], F32, tag=f"pV{c}",
                               name=f"pV{c}") for c in range(4)]
                for kt in range(KT):
                    xg = stage.tile([128, 512], R, tag="xg")
                    eng = nc.gpsimd if kt % 2 == 0 else nc.scalar
                    eng.dma_start(
                        out=xg[:], in_=xtb[kt * 128:(kt + 1) * 128,
                                           c0:c0 + 512])
                    for kv in range(KV):
                        nc.tensor.matmul(
                            pKT[kv][:], wk_r[:, kt, kv * E:(kv + 1) * E],
                            xg[:], start=(kt == 0), stop=(kt == KT - 1))
                    for c in range(4):
                        nc.tensor.matmul(
                            pV[c][:], xg[:, c * 128:(c + 1) * 128],
                            wv_r[:, kt, :], start=(kt == 0),
                            stop=(kt == KT - 1))
                for kv in range(KV):
                    nc.vector.tensor_scalar(
                        kT[kv][:, c0:c0 + 512], pKT[kv][:],
                        bk_t[:, kv:kv + 1], None, op0=ALU.add)
                for c in range(4):
                    nc.vector.tensor_add(vN[:, g4 * 4 + c, :], pV[c][:],
                                         bv_t[:])
                if g4 == 0:
                    emit_xtq_loads()

        # ====== Phase 2a: Q^T projection, heads 0..7 =======================
        # (heads 8..15 are projected inside phase 3a as PE filler work)
        with ExitStack() as ph:
            stage = ph.enter_context(tc.tile_pool(name="stage2", bufs=8))
            ps2 = ph.enter_context(tc.tile_pool(name="ps2", bufs=1,
                                                space="PSUM"))
            HB = 8
            pqs = [ps2.tile([E, QR], F32, tag=f"pq{hh}", name=f"pq{hh}")
                   for hh in range(HB)]
            for kt in range(KT):
                wqs = stage.tile([128, HB, E], BF, tag="wqs")
                nc.sync.dma_start(
                    out=wqs[:],
                    in_=wqb[kt * 128:(kt + 1) * 128, 0:HB * E]
                    .rearrange("p (h e) -> p h e", h=HB))
                for hh in range(HB):
                    nc.tensor.matmul(
                        pqs[hh][:], wqs[:, hh, :], xtq_r[:, kt, :],
                        start=(kt == 0), stop=(kt == KT - 1))
            for hh in range(HB):
                nc.vector.tensor_scalar(
                    qT[hh][:], pqs[hh][:], bq_t[:, hh:hh + 1], None,
                    op0=ALU.add)

        # ================= Phase 3: attention ==============================
        # The exp stream on the scalar engine is the phase bottleneck, so
        # the PE is kept busy (and HAM-warm) with interleaved filler:
        #   heads 0..7:  Q-projection of head 8+h (2 matmuls / kb slot)
        #   heads 8..15: out-proj partial sums over heads 0..7 into pA
        with ExitStack() as ph:
            ps_s = ph.enter_context(tc.tile_pool(name="pss", bufs=2,
                                                 space="PSUM"))
            ps_c = ph.enter_context(tc.tile_pool(name="psc", bufs=2,
                                                 space="PSUM"))
            ps_l = ph.enter_context(tc.tile_pool(name="psl", bufs=1,
                                                 space="PSUM"))
            ps_f = ph.enter_context(tc.tile_pool(name="psf", bufs=1,
                                                 space="PSUM"))
            exp_p = ph.enter_context(tc.tile_pool(name="expp", bufs=3))
            lso = ph.enter_context(tc.tile_pool(name="lso", bufs=2))
            stage3 = ph.enter_context(tc.tile_pool(name="stage3", bufs=2))
            woAp = ph.enter_context(tc.tile_pool(name="woAp", bufs=1))

            # wo rows for heads 0..7 (the 3b filler's moving operand)
            woA = woAp.tile([128, H // 2, D], BF)
            for hh in range(H // 2):
                nc.scalar.dma_start(out=woA[:, hh, :],
                                    in_=wob[hh * E:(hh + 1) * E, :])

            wq2 = {}

            def emit_wq2_dma(h2):
                if h2 >= H:
                    return
                w = stage3.tile([128, KT, E], BF, tag="wq2",
                                name=f"wq2_{h2}")
                nc.sync.dma_start(
                    out=w[:],
                    in_=wqb[:, h2 * E:(h2 + 1) * E]
                    .rearrange("(kt p) e -> p kt e", p=128))
                wq2[h2] = w

            emit_wq2_dma(8)
            fill_state = {"pq2": None, "pys": None, "kt": 0, "gs": 0}
            # filler matmuls per kb slot, weighted toward the small-qc
            # (PE-light) late key blocks so exp latency stays hidden
            FILL_W = [1, 1, 1, 2, 2, 2, 3, 4]
            assert sum(FILL_W) == 16

            def emit_filler(h, kb):
                n = FILL_W[kb]
                if h < H // 2:
                    h2 = 8 + h
                    if kb == 0:
                        fill_state["pq2"] = ps_f.tile([E, QR], F32,
                                                      tag="fill",
                                                      name=f"pq2_{h2}")
                        fill_state["kt"] = 0
                    pq2 = fill_state["pq2"]
                    for _ in range(n):
                        kt = fill_state["kt"]
                        fill_state["kt"] += 1
                        nc.tensor.matmul(
                            pq2[:], wq2[h2][:, kt, :], xtq_r[:, kt, :],
                            start=(kt == 0), stop=(kt == KT - 1))
                    if kb == NKB - 1:
                        nc.vector.tensor_scalar(
                            qT[h2][:], pq2[:], bq_t[:, h2:h2 + 1], None,
                            op0=ALU.add)
                else:
                    for _ in range(n):
                        gs = fill_state["gs"]
                        fill_state["gs"] += 1
                        u, step = divmod(gs, 8)
                        oc, rt = divmod(u, RT)
                        if step == 0:
                            fill_state["pys"] = ps_f.tile([128, OC], F32,
                                                          tag="fill",
                                                          name=f"pysA_{u}")
                        pys = fill_state["pys"]
                        nc.tensor.matmul(
                            pys[:], ctxT[step][:, rt * 128:(rt + 1) * 128],
                            woA[:, step, oc * OC:(oc + 1) * OC],
                            start=(step == 0), stop=(step == 7))
                        if step == 7:
                            nc.vector.tensor_copy(
                                pA[rt][:, oc * OC:(oc + 1) * OC], pys[:])

            for h in range(H):
                kv = h % KV
                if h < H // 2:
                    emit_wq2_dma(9 + h)
                pctx = ps_c.tile([E, QR], F32, tag="pctx")
                pl = ps_l.tile([2, QR], F32, tag="pl")
                eSs = [None] * NKB
                q0s = [None] * NKB

                def emit_pl_ctx(kb):
                    eS, q0 = eSs[kb], q0s[kb]
                    qc = QR - q0
                    first = (kb == 0)
                    last = (kb == NKB - 1)
                    for st in range(ST):
                        k0 = (kb * ST + st)
                        nc.tensor.matmul(
                            pl[:, q0:], ones_kb[:, :], eS[:, st, :qc],
                            start=first and st == 0,
                            stop=last and st == ST - 1,
                            skip_group_check=True)
                        nc.tensor.matmul(
                            pctx[:, q0:], vN[:, k0, kv * E:(kv + 1) * E],
                            eS[:, st, :qc],
                            start=first and st == 0,
                            stop=last and st == ST - 1,
                            skip_group_check=True)

                for kb in range(NKB):
                    q0 = g * kb
                    qc = QR - q0
                    q0s[kb] = q0
                    pS = ps_s.tile([KSS, ST, QR], F32, tag="pS")
                    for st in range(ST):
                        k0 = kb * cfg.KB + st * KSS
                        nc.tensor.matmul(pS[:, st, :qc],
                                         kT[kv][:, k0:k0 + KSS],
                                         qT[h][:, q0:], start=True, stop=True)
                    emit_filler(h, kb)
                    # pipeline: previous block's pl/pctx go behind these
                    # scores so the PE isn't blocked on this block's exp.
                    if kb > 0:
                        emit_pl_ctx(kb - 1)
                    # additive causal mask (0/-1e9) on the diagonal window
                    nc.vector.tensor_add(pS[:, :, :g], pS[:, :, :g], mk_t[:])
                    eS = exp_p.tile([KSS, ST, QR], BF, tag="eS")
                    nc.scalar.activation(eS[:, :, :qc], pS[:, :, :qc],
                                         AF.Exp, scale=inv_sqrt_e)
                    eSs[kb] = eS
                emit_pl_ctx(NKB - 1)

                l2f = lso.tile([1, QR], F32, tag="l2f")
                nc.vector.reciprocal_approx_fast(l2f[:], pl[:1, :])
                rb = lso.tile([128, QR], F32, tag="rb")
                nc.gpsimd.partition_broadcast(rb[:], l2f[:])
                nc.vector.tensor_mul(ctxT[h][:], pctx[:], rb[:])

        xtq_stack.close()
        kvq_stack.close()

        # ===== Phase 4+5: out-proj (heads 8..15) + GELU + residual + LN ====
        # rt-outer with wo fully resident: each row tile's LayerNorm +
        # output DMA drains while the next row tile's matmuls run.
        with ExitStack() as ph:
            wop = ph.enter_context(tc.tile_pool(name="wop", bufs=1))
            rfp = ph.enter_context(tc.tile_pool(name="rfp", bufs=2))
            ps_y = ph.enter_context(tc.tile_pool(name="psy", bufs=4,
                                                 space="PSUM"))
            ep = ph.enter_context(tc.tile_pool(name="epp", bufs=3))
            stat = ph.enter_context(tc.tile_pool(name="stat", bufs=1))
            gbp = ph.enter_context(tc.tile_pool(name="gbp", bufs=1))

            woB = wop.tile([128, H // 2, D], BF)
            # oc-major chunk order so (rt0, oc0) only waits for the first MB
            for oc0 in range(NOC):
                for pc in range(2):
                    h0 = H // 2 + pc * 4
                    nc.sync.dma_start(
                        out=woB[:, pc * 4:(pc + 1) * 4,
                                oc0 * OC:(oc0 + 1) * OC],
                        in_=wob[h0 * E:(h0 + 4) * E, oc0 * OC:(oc0 + 1) * OC]
                        .rearrange("(h p) c -> p h c", p=128))
            xqr = [ep.tile([128, D], F32, tag=f"xqr{rt}", name=f"xqr{rt}",
                           bufs=1) for rt in range(RT)]
            for rt in range(RT):
                nc.scalar.dma_start(out=xqr[rt][:],
                                    in_=xq[rt * 128:(rt + 1) * 128, :])
            if not cfg.trivial_affine:
                bo_f = gbp.tile([128, D], F32)
                gm_f = gbp.tile([128, D], F32)
                bt_f = gbp.tile([128, D], F32)
                nc.scalar.dma_start(out=bo_f[:], in_=bob[:])
                nc.scalar.dma_start(out=gm_f[:], in_=gmb[:])
                nc.scalar.dma_start(out=bt_f[:], in_=btb[:])

            for rt in range(RT):
                r_full = rfp.tile([128, D], F32, tag="rf")
                bna = stat.tile([128, NOC, 6], F32, tag="bna")
                for oc in range(NOC):
                    pys = ps_y.tile([128, OC], F32, tag="pys")
                    for hh in range(H // 2):
                        nc.tensor.matmul(
                            pys[:],
                            ctxT[H // 2 + hh][:, rt * 128:(rt + 1) * 128],
                            woB[:, hh, oc * OC:(oc + 1) * OC],
                            start=(hh == 0), stop=(hh == H // 2 - 1))
                    tb = ep.tile([128, OC], F32, tag="tb")
                    nc.vector.tensor_add(tb[:], pys[:],
                                         pA[rt][:, oc * OC:(oc + 1) * OC])
                    if not cfg.trivial_affine:
                        tb2 = ep.tile([128, OC], F32, tag="tb2")
                        nc.vector.tensor_add(
                            tb2[:], tb[:], bo_f[:, oc * OC:(oc + 1) * OC])
                        tb = tb2
                    t2 = ep.tile([128, OC], F32, tag="t2")
                    nc.scalar.activation(t2[:], tb[:], act_fn)
                    rch = r_full[:, oc * OC:(oc + 1) * OC]
                    nc.vector.tensor_add(rch, t2[:],
                                         xqr[rt][:, oc * OC:(oc + 1) * OC])
                    nc.vector.bn_stats(bna[:, oc, :], rch)
                # stats complete for this row tile: LN + drain now, while
                # the next row tile's matmuls occupy the PE.
                mv = stat.tile([128, 2], F32, tag="mv")
                nc.vector.bn_aggr(mv[:], bna[:])
                v_e = stat.tile([128, 1], F32, tag="ve")
                nc.vector.tensor_scalar_add(v_e[:], mv[:, 1:2], 1e-5)
                # Newton rsqrt on vector only (no ACT table switch):
                # y0 = 1.09545 - 0.1895*v, then 4x y *= 1.5 - 0.5*v*y^2
                y = stat.tile([128, 1], F32, tag="y")
                nc.vector.tensor_scalar(y[:], v_e[:], -0.1895,
                                        1.09545, op0=ALU.mult, op1=ALU.add)
                for _ in range(4):
                    h2t = stat.tile([128, 1], F32, tag="h2t")
                    nc.vector.tensor_mul(h2t[:], y[:], y[:])
                    nc.vector.tensor_mul(h2t[:], h2t[:], v_e[:])
                    nc.vector.tensor_scalar(h2t[:], h2t[:], -0.5, 1.5,
                                            op0=ALU.mult, op1=ALU.add)
                    nc.vector.tensor_mul(y[:], y[:], h2t[:])
                nmr = stat.tile([128, 1], F32, tag="nmr")
                nc.vector.scalar_tensor_tensor(
                    nmr[:], mv[:, 0:1], -1.0, y[:],
                    op0=ALU.mult, op1=ALU.mult)
                yfull = ep.tile([128, D], F32, tag="yfull", bufs=2)
                for c in range(NOC):
                    slc = slice(c * OC, (c + 1) * OC)
                    if cfg.trivial_affine:
                        nc.vector.tensor_scalar(
                            yfull[:, slc], r_full[:, slc], y[:], nmr[:],
                            op0=ALU.mult, op1=ALU.add)
                    else:
                        yf = ep.tile([128, OC], F32, tag="yf")
                        nc.vector.tensor_scalar(
                            yf[:], r_full[:, slc], y[:], nmr[:],
                            op0=ALU.mult, op1=ALU.add)
                        y2 = ep.tile([128, OC], F32, tag="y2")
                        nc.vector.tensor_mul(y2[:], yf[:], gm_f[:, slc])
                        nc.vector.tensor_add(yfull[:, slc], y2[:],
                                             bt_f[:, slc])
                # two half-width DMAs on different queues: 4KB-contiguous
                # rows, twice the drain parallelism
                e0, e1 = ((nc.gpsimd, nc.sync), (nc.scalar, nc.gpsimd),
                          (nc.sync, nc.scalar), (nc.gpsimd, nc.sync))[rt]
                hD = D // 2
                e0.dma_start(out=out[rt * 128:(rt + 1) * 128, :hD],
                             in_=yfull[:, :hD])
                e1.dma_start(out=out[rt * 128:(rt + 1) * 128, hD:],
                             in_=yfull[:, hD:])

    nc.finalize()
    return nc


# ---------------------------------------------------------------------------
# host-side staging + sharding
# ---------------------------------------------------------------------------

def _bf16(a):
    import ml_dtypes
    return np.ascontiguousarray(np.asarray(a, np.float32)).astype(
        ml_dtypes.bfloat16)


def build_mask01(cfg: Cfg, j: int):
    # mk01[c, r] = 0 iff key (c = st*128 + k) is visible to the r-th query
    # of the diagonal block (c <= 64*j + r), else -1e9; same for every kb.
    c = np.arange(cfg.KB)[:, None]
    r = np.arange(cfg.g)[None, :]
    return np.where(c <= j * cfg.g + r, 0.0, -1.0e9).astype(np.float32)


def q_rows(cfg: Cfg, j: int):
    g = cfg.g
    return np.concatenate(
        [np.arange((j + 4 * i) * g, (j + 4 * i + 1) * g) for i in range(8)])


def make_in_map(cfg: Cfg, shared, xb_T_f32, xb_f32, j):
    rows = q_rows(cfg, j)
    return dict(
        shared,
        xtb=xb_T_f32,
        xtqb=np.ascontiguousarray(_bf16(xb_T_f32[:, rows])),
        xq=np.ascontiguousarray(xb_f32[rows]),
        mk01=build_mask01(cfg, j),
    )


def make_shared(cfg: Cfg, Wq, bq, Wk, bk, Wv, bv, Wo, bo, gamma, beta):
    H, KV, E, D = cfg.H, cfg.KV, cfg.E, cfg.D
    shared = {
        "wqb": _bf16(Wq),
        "wkb": np.ascontiguousarray(Wk, dtype=np.float32),
        "wvb": np.ascontiguousarray(Wv, dtype=np.float32),
        "wob": _bf16(Wo),
        "bqT": np.ascontiguousarray(
            np.asarray(bq, np.float32).reshape(H, E).T),
        "bkT": np.ascontiguousarray(
            np.asarray(bk, np.float32).reshape(KV, E).T),
        "bvb": np.ascontiguousarray(
            np.broadcast_to(np.asarray(bv, np.float32), (128, KV * E))),
    }
    if not cfg.trivial_affine:
        shared["bob"] = np.ascontiguousarray(
            np.broadcast_to(np.asarray(bo, np.float32), (128, D)))
        shared["gmb"] = np.ascontiguousarray(
            np.broadcast_to(np.asarray(gamma, np.float32), (128, D)))
        shared["btb"] = np.ascontiguousarray(
            np.broadcast_to(np.asarray(beta, np.float32), (128, D)))
    return shared


def assemble(cfg: Cfg, results, B):
    out = np.empty((B, cfg.L, cfg.D), np.float32)
    for core in range(4 * B):
        b, j = divmod(core, 4)
        out[b, q_rows(cfg, j)] = results[core]["out"]
    return out


_NC_CACHE = {}


def kernel(x, Wq, bq, Wk, bk, Wv, bv, Wo, bo, gamma, beta):
    from concourse.bass_utils import run_bass_kernel_spmd

    trivial = bool(
        np.all(np.asarray(gamma) == 1.0) and np.all(np.asarray(beta) == 0.0)
        and np.all(np.asarray(bo) == 0.0))
    cfg = Cfg(trivial_affine=trivial)
    if cfg not in _NC_CACHE:
        _NC_CACHE[cfg] = build_program(cfg)
    nc = _NC_CACHE[cfg]
    shared = make_shared(cfg, Wq, bq, Wk, bk, Wv, bv, Wo, bo, gamma, beta)
    x = np.asarray(x, np.float32)
    xT = [np.ascontiguousarray(x[b].T) for b in range(2)]
    in_maps = [make_in_map(cfg, shared, xT[core // 4], x[core // 4],
                           core % 4)
               for core in range(8)]
    res = run_bass_kernel_spmd(nc, in_maps, list(range(8)))
    return assemble(cfg, res.results, 2)
